# revision 1
# baseline (speedup 1.0000x reference)
"""2-layer GCN (PyG GCNConv x2 + sigmoid) on 8 TRN2 NeuronCores, single fused NEFF.

Design (memory-regime, gather-bound):
- All GCN normalization is folded out of the per-edge path:
  out = sigmoid(dinv_dst * segsum(M'[src]) + b), with M' = W1^T @ (x * dinv)^T
  built on-device by the PE. Per-edge work is pure gather + sum.
- Layer 1: dst-sharded across cores; feature-major source tables built in 4
  pipelined node-quarters (12.5K nodes each, fits int16 gather indices);
  GPSIMD ap_gather streams per-edge rows (~1.4ns/edge); exact segment sums
  via degree-ladder tensor_reduce with compile-time-uniform budgets across
  cores; perm-gather reassembles node order; finalize is sliced to overlap
  the last quarter's assembly.
- z' = h'@W2 shards are AllGathered on-device (DRAM bounce, Shared output);
  falls back to a two-launch host-crossing variant if collectives fail.
- Layer 2: scalar gathers use the 16-partition-group structure of ap_gather
  (8 independent edge groups per instruction -> 8x throughput); tiny tables
  live on stride-16 partitions only.
- Host does only index/layout preprocessing: degrees, ladder packing with
  degree bucketing, a src-table permutation that dealigns self-loops (keeps
  shared max-over-core budgets ~13% instead of ~34% over actual edges),
  int16 index wrapping (16B-aligned slices), output unpermutation.
"""

import sys

sys.path.insert(0, "/opt/trn_rl_repo")
import numpy as np
from contextlib import ExitStack

from concourse import bacc, mybir
from concourse.tile import TileContext
from concourse.bass_utils import run_bass_kernel_spmd

MEASURE = False  # when True, run the cost-model simulator and fill LAST_SIM_NS
LAST_SIM_NS = None

N = 50000
E = 800000
F = 128
P = 128
NCORES = 8
NSH = N // NCORES  # 6250 dst nodes per core
NQ = 4  # src quarters
QN = N // NQ  # 12500 nodes per quarter
QCOLS = 12800  # quarter table cols: [zero, 12500 nodes, pad] -> 25 chunks of 512
MMCH = 512  # matmul chunk
G1 = 2560  # k1 gather chunk (slots; /16 cols stays 16B-aligned)
NGROUP = 8  # k2: 16-partition groups


def _wrap16(idx_flat):
    """logical sequence -> [16, n/16] wrapped layout"""
    n = idx_flat.shape[0]
    assert n % 16 == 0
    return np.ascontiguousarray(idx_flat.reshape(n // 16, 16).T)


def _pad16(n, mult=16):
    return ((n + mult - 1) // mult) * mult


def _pad128(n):
    return ((n + 127) // 128) * 128


def host_prep(x, edge_index, W1, b1, W2, b2):
    """All index/layout preprocessing. Returns per-core input maps + metadata."""
    src = np.concatenate([edge_index[0], np.arange(N, dtype=np.int64)]).astype(np.int32)
    dst = np.concatenate([edge_index[1], np.arange(N, dtype=np.int64)]).astype(np.int32)
    deg = np.bincount(dst, minlength=N).astype(np.float32)
    dinv = 1.0 / np.sqrt(np.maximum(deg, 1e-12))
    dinv[deg <= 0] = 0.0

    # Permute the src-table node order so each node's self-loop lands in a
    # pseudo-random quarter: keeps per-(core,quarter) degree distributions
    # aligned across cores, which keeps the shared max-over-cores ladder
    # budgets tight. dst-side layout is unaffected.
    psrc = np.random.default_rng(12345).permutation(N)  # node -> table position
    pinv = np.argsort(psrc)  # table position -> node

    # xT_pre[f, pos] = x[node, f] * dinv[node]; layout per quarter: [zero, nodes, pad]
    xtp = (x * dinv[:, None]).T.astype(np.float32)[:, pinv]  # [128, N] position order
    xt = np.zeros((P, NQ * QCOLS), dtype=np.float32)
    for q in range(NQ):
        xt[:, q * QCOLS + 1 : q * QCOLS + 1 + QN] = xtp[:, q * QN : (q + 1) * QN]

    core = dst // NSH  # [Etot]
    dstl = dst % NSH
    pos = psrc[src]
    quarter = pos // QN
    srcl = (pos % QN).astype(np.int32) + 1  # 0 = zero col

    # per (core, quarter): kappa counts per local dst node
    kap = np.zeros((NCORES, NQ, NSH), dtype=np.int32)
    for c in range(NCORES):
        mc = core == c
        for q in range(NQ):
            m = mc & (quarter == q)
            kap[c, q] = np.bincount(dstl[m], minlength=NSH)

    kmax = int(kap.max())
    # bucketed ladder: exact for small degrees, coarse above (pools the sparse
    # tail so the max-over-cores budget inflation stays small)
    lut = np.arange(kmax + 1)
    for kk in range(9, kmax + 1):
        for bb in (10, 12, 14, 17, 21, 26, 32, 40, 48, 64, 96, 128, 192, 256):
            if kk <= bb:
                lut[kk] = bb
                break
    kapb = lut[kap]
    # ladder budgets per quarter: n_b = max over cores of #nodes with bucket==b
    budgets = []  # budgets[q] = {bucket: n_b}
    for q in range(NQ):
        b = {}
        for k in np.unique(kapb[:, q, :]):
            k = int(k)
            if k == 0:
                continue
            nk = int((kapb[:, q, :] == k).sum(axis=1).max())
            if nk > 0:
                b[k] = nk
        budgets.append(b)

    # pack ladder rows into G1-slot chunks; shared layout per quarter
    # descriptors: (chunk_idx, slot_off_in_chunk, n_rows, k, accp_col_off)
    layouts = []  # layouts[q] = (n_chunks, [descr], accp_cols, {k: col_off})
    for q in range(NQ):
        descr = []
        kbase = {}
        col = 1  # col 0 = zero col
        ch, off = 0, 0
        for k in sorted(budgets[q]):
            nk = budgets[q][k]
            kbase[k] = col
            left = nk
            while left > 0:
                fit = min(left, (G1 - off) // k)
                if fit == 0:
                    ch += 1
                    off = 0
                    fit = min(left, G1 // k)
                descr.append((ch, off, fit, k, col))
                off += fit * k
                col += fit
                left -= fit
            # next k continues filling same chunk
        n_chunks = ch + 1
        layouts.append((n_chunks, descr, col, kbase))

    SQ = [layouts[q][0] * G1 for q in range(NQ)]  # slots per quarter
    PQ = max(layouts[q][2] for q in range(NQ))  # accP col budget
    PQ = _pad16(PQ)

    # build per-core slot-index arrays + perms
    eidx = np.zeros((NCORES, sum(SQ)), dtype=np.int16)
    PERM_NI = _pad128(NSH)  # 6272; multiple of 128 so idx slices stay 16B-aligned
    perms = np.zeros((NCORES, NQ, PERM_NI), dtype=np.int16)
    order = np.lexsort((dstl, quarter, core))  # edges grouped by (core, quarter, dst)
    so, do_, qo, co = srcl[order], dstl[order], quarter[order], core[order]
    for c in range(NCORES):
        qbase = 0
        for q in range(NQ):
            m = (co == c) & (qo == q)
            s_cq, d_cq = so[m], do_[m]  # sorted by dst
            kv = kap[c, q]
            kvb = lut[kv]
            # nodes with kappa>0, bucket-grouped: rank within bucket-section
            nodes = np.nonzero(kv)[0]
            kn = kv[nodes]  # actual degree (slots filled)
            knb = kvb[nodes]  # bucket (row width)
            nd_order = np.lexsort((nodes, knb))  # sort nodes by (bucket, node)
            nodes_s = nodes[nd_order]
            kn_s = kn[nd_order]
            knb_s = knb[nd_order]
            # row start slot for each node, following the shared layout
            _, descr, _, kbase = layouts[q]
            # per-bucket: rank of node among same-bucket nodes
            rank = np.zeros(len(nodes_s), dtype=np.int64)
            colof = np.zeros(len(nodes_s), dtype=np.int64)
            for k in np.unique(knb_s):
                mk = knb_s == k
                rank[mk] = np.arange(mk.sum())
                colof[mk] = kbase[int(k)]
            node_col = colof + rank  # accP column of each node
            perms[c, q, : len(nodes)] = 0
            pm = np.zeros(NSH, dtype=np.int16)
            pm[nodes_s] = node_col.astype(np.int16)
            perms[c, q, :NSH] = pm
            # slot position of each (row=node_col, lane): need chunk/slot map per accP col
            col2slot = np.full(layouts[q][2], -1, dtype=np.int64)
            for ch, off, n_rows, k, col in descr:
                cols = np.arange(n_rows)
                col2slot[col + cols] = ch * G1 + off + cols * k
            # edges of node appear consecutively (sorted by dst within (c,q))
            # slot of edge j of node n = col2slot[node_col[n]] + j
            # build via repeat
            starts = col2slot[node_col]
            eslots = np.repeat(starts, kn_s) + _concat_aranges(kn_s)
            # values: srcl of edges, grouped per node ascending-dst...
            # s_cq is sorted by dst; nodes_s is sorted by (k,node) -> reorder edges
            edge_node_ptr = np.zeros(NSH + 1, dtype=np.int64)
            edge_node_ptr[1:] = np.cumsum(kv)
            ev = np.concatenate(
                [s_cq[edge_node_ptr[n] : edge_node_ptr[n + 1]] for n in nodes_s]
            ) if len(nodes_s) else np.zeros(0, dtype=np.int32)
            eidx[c, qbase + eslots] = ev.astype(np.int16)
            qbase += SQ[q]

    # wrap idx arrays
    eidx_w = np.zeros((NCORES, P, sum(SQ) // 16), dtype=np.int16)
    perm_w = np.zeros((NCORES, P, NQ * (PERM_NI // 16)), dtype=np.int16)
    for c in range(NCORES):
        eidx_w[c] = np.tile(_wrap16(eidx[c]), (NGROUP, 1))
        pw = np.concatenate([_wrap16(perms[c, q]) for q in range(NQ)], axis=1)
        perm_w[c] = np.tile(pw, (NGROUP, 1))

    dinvb = np.stack([np.tile(dinv[c * NSH : (c + 1) * NSH], (P, 1)) for c in range(NCORES)])

    meta = dict(layouts=layouts, SQ=SQ, PQ=PQ, PERM_NI=PERM_NI, dinv=dinv)
    k1_inputs = []
    for c in range(NCORES):
        k1_inputs.append(
            {
                "xt": xt,
                "w1": W1.astype(np.float32),
                "b1": b1.astype(np.float32).reshape(P, 1),
                "w2": W2.astype(np.float32),
                "eidx": np.ascontiguousarray(eidx_w[c]),
                "perm": np.ascontiguousarray(perm_w[c]),
                "dinvb": np.ascontiguousarray(dinvb[c].astype(np.float32)),
            }
        )
    return k1_inputs, meta, (src, dst, dinv)


def _concat_aranges(lens):
    """[2,3] -> [0,1,0,1,2]"""
    if len(lens) == 0:
        return np.zeros(0, dtype=np.int64)
    total = int(lens.sum())
    out = np.ones(total, dtype=np.int64)
    ends = np.cumsum(lens)
    out[0] = 0
    out[ends[:-1]] = -(lens[:-1] - 1)
    return np.cumsum(out)


def build_k1(meta, debug_acc=False):
    layouts, SQ, PQ, PERM_NI = meta["layouts"], meta["SQ"], meta["PQ"], meta["PERM_NI"]
    nc = bacc.Bacc(None, target_bir_lowering=False)
    f32, i16 = mybir.dt.float32, mybir.dt.int16
    xt_d = nc.dram_tensor("xt", [P, NQ * QCOLS], f32, kind="ExternalInput")
    w1_d = nc.dram_tensor("w1", [P, P], f32, kind="ExternalInput")
    b1_d = nc.dram_tensor("b1", [P, 1], f32, kind="ExternalInput")
    w2_d = nc.dram_tensor("w2", [P, 1], f32, kind="ExternalInput")
    eidx_d = nc.dram_tensor("eidx", [P, sum(SQ) // 16], i16, kind="ExternalInput")
    perm_d = nc.dram_tensor("perm", [P, NQ * (PERM_NI // 16)], i16, kind="ExternalInput")
    dinvb_d = nc.dram_tensor("dinvb", [P, NSH], f32, kind="ExternalInput")
    zout_d = nc.dram_tensor("zout", [1, NSH], f32, kind="ExternalOutput")
    accout_d = (
        nc.dram_tensor("accout", [P, NSH], f32, kind="ExternalOutput") if debug_acc else None
    )

    with ExitStack() as ctx:
        tc = ctx.enter_context(TileContext(nc))
        cpool = ctx.enter_context(tc.tile_pool(name="cpool", bufs=1))
        apool = ctx.enter_context(tc.tile_pool(name="apool", bufs=1))
        w1 = cpool.tile([P, P], f32)
        b1 = cpool.tile([P, 1], f32)
        w2 = cpool.tile([P, 1], f32)
        eidx = cpool.tile([P, sum(SQ) // 16], i16)
        perm = cpool.tile([P, NQ * (PERM_NI // 16)], i16)
        acc = apool.tile([P, NSH], f32)
        accp = apool.tile([P, PQ], f32)
        nc.sync.dma_start(out=w1[:], in_=w1_d[:])
        nc.sync.dma_start(out=b1[:], in_=b1_d[:])
        nc.sync.dma_start(out=w2[:], in_=w2_d[:])
        nc.sync.dma_start(out=eidx[:], in_=eidx_d[:])
        nc.sync.dma_start(out=perm[:], in_=perm_d[:])
        nc.vector.memset(accp[:, 0:1], 0.0)

        with (
            tc.tile_pool(name="tabs", bufs=2) as tabs,
            tc.tile_pool(name="xpool", bufs=3) as xpool,
            tc.tile_pool(name="gpool", bufs=2) as gpool,
            tc.tile_pool(name="pspool", bufs=2, space="PSUM") as pspool,
        ):
            sq_base = 0
            for q in range(NQ):
                n_chunks, descr, _, _ = layouts[q]
                tab = tabs.tile([P, QCOLS], f32, tag="tab")
                # build quarter table: tab = W1^T @ xt[:, quarter]
                XB = 2 * MMCH  # 1024-col x loads (524KB DMAs)
                for x0 in range(0, QCOLS, XB):
                    xw = min(XB, QCOLS - x0)
                    xc = xpool.tile([P, XB], f32, tag="x")
                    nc.sync.dma_start(
                        out=xc[:, :xw], in_=xt_d[:, q * QCOLS + x0 : q * QCOLS + x0 + xw]
                    )
                    for m0 in range(0, xw, MMCH):
                        ps = pspool.tile([P, MMCH], f32, tag="ps")
                        nc.tensor.matmul(ps[:], w1[:], xc[:, m0 : m0 + MMCH], start=True, stop=True)
                        nc.scalar.activation(
                            tab[:, x0 + m0 : x0 + m0 + MMCH], ps[:],
                            mybir.ActivationFunctionType.Copy,
                        )
                # gather + ladder reduces
                by_chunk = {}
                for d in descr:
                    by_chunk.setdefault(d[0], []).append(d)
                for ch in range(n_chunks):
                    g = gpool.tile([P, G1], f32, tag="g")
                    i0 = (sq_base + ch * G1) // 16
                    nc.gpsimd.ap_gather(
                        g[:], tab[:], eidx[:, i0 : i0 + G1 // 16],
                        channels=P, num_elems=QCOLS, d=1, num_idxs=G1,
                    )
                    for (_, off, n_rows, k, col) in by_chunk.get(ch, []):
                        nc.vector.tensor_reduce(
                            accp[:, col : col + n_rows],
                            g[:, off : off + n_rows * k].rearrange(
                                "p (a b) -> p a b", a=n_rows, b=k
                            ),
                            axis=mybir.AxisListType.X, op=mybir.AluOpType.add,
                        )
                # assemble: acc (+)= accp[perm] in G1-col pieces
                pbase = q * (PERM_NI // 16)
                for s0 in range(0, PERM_NI, G1):
                    w = min(G1, PERM_NI - s0)
                    w = min(w, NSH - s0) if s0 < NSH else 0
                    if w <= 0:
                        break
                    wp = _pad16(w)
                    t = gpool.tile([P, G1], f32, tag="g")
                    nc.gpsimd.ap_gather(
                        t[:, :wp], accp[:], perm[:, pbase + s0 // 16 : pbase + (s0 + wp) // 16],
                        channels=P, num_elems=PQ, d=1, num_idxs=wp,
                    )
                    if q == 0:
                        nc.scalar.activation(
                            acc[:, s0 : s0 + w], t[:, :w], mybir.ActivationFunctionType.Copy
                        )
                    else:
                        nc.vector.tensor_add(acc[:, s0 : s0 + w], acc[:, s0 : s0 + w], t[:, :w])
                sq_base += SQ[q]

        if debug_acc:
            nc.sync.dma_start(out=accout_d[:], in_=acc[:])
        # finalize: h' = dinv*sigmoid(dinv*acc + b1); z' = W2^T @ h'
        with (
            tc.tile_pool(name="fin", bufs=1) as fin,
            tc.tile_pool(name="zpspool", bufs=2, space="PSUM") as zps,
        ):
            dinvb = fin.tile([P, NSH], f32)
            zrow = fin.tile([1, NSH], f32)
            nc.sync.dma_start(out=dinvb[:], in_=dinvb_d[:])
            nc.vector.tensor_mul(acc[:], acc[:], dinvb[:])
            nc.scalar.activation(acc[:], acc[:], mybir.ActivationFunctionType.Sigmoid, bias=b1[:, 0:1])
            nc.vector.tensor_mul(acc[:], acc[:], dinvb[:])
            for m0 in range(0, NSH, MMCH):
                w = min(MMCH, NSH - m0)
                ps = zps.tile([1, MMCH], f32, tag="zps")
                nc.tensor.matmul(ps[:, :w], w2[:], acc[:, m0 : m0 + w], start=True, stop=True)
                nc.scalar.activation(zrow[:, m0 : m0 + w], ps[:, :w], mybir.ActivationFunctionType.Copy)
            nc.sync.dma_start(out=zout_d[:], in_=zrow[:])
    nc.finalize()
    return nc


def host_prep_k2(zfull, src, dst, dinv, b2):
    """Layer-2: scalar gather with 8 independent 16-partition groups."""
    core = dst // NSH
    dstl = dst % NSH
    quarter = src // QN
    srcl = (src % QN).astype(np.int32) + 1
    grp = dstl % NGROUP  # node -> group

    # kappa per (core, quarter, group, node-within-group)
    GN = NSH // NGROUP  # 781.25 -> careful: use dstl//NGROUP as local id (0..781)
    gid = dstl // NGROUP
    GNN = (NSH + NGROUP - 1) // NGROUP  # 782
    kap = np.zeros((NCORES, NQ, NGROUP, GNN), dtype=np.int32)
    for c in range(NCORES):
        mc = core == c
        for q in range(NQ):
            mq = mc & (quarter == q)
            for g in range(NGROUP):
                m = mq & (grp == g)
                kap[c, q, g] = np.bincount(gid[m], minlength=GNN)

    kmax = int(kap.max())
    lut = np.arange(kmax + 1)
    for kk in range(5, kmax + 1):
        for bb in (6, 8, 10, 12, 15, 19, 24, 30, 38, 48, 64, 96, 128, 192, 256):
            if kk <= bb:
                lut[kk] = bb
                break
    kapb = lut[kap]
    budgets, layouts = [], []
    for q in range(NQ):
        b = {}
        for k in np.unique(kapb[:, q, :, :]):
            k = int(k)
            if k == 0:
                continue
            nk = int((kapb[:, q, :, :] == k).sum(axis=2).max())
            if nk > 0:
                b[k] = nk
        budgets.append(b)
        descr, kbase = [], {}
        col = 1
        slots = 0
        for k in sorted(b):
            kbase[k] = col
            descr.append((slots, b[k], k, col))
            slots += b[k] * k
            col += b[k]
        slots = _pad128(slots)
        layouts.append((slots, descr, col, kbase))

    P2 = _pad128(max(l[2] for l in layouts) if layouts else 128)
    SQ2 = [l[0] for l in layouts]

    # z tables: [8, QCOLS2] per quarter, col0=0
    QC2 = QN + 1
    ztab = None
    if zfull is not None:
        ztab = np.zeros((NQ, NGROUP, QC2), dtype=np.float32)
        for q in range(NQ):
            ztab[q, :, 1:] = zfull[q * QN : (q + 1) * QN][None, :]

    eidx2 = np.zeros((NCORES, NGROUP, sum(SQ2)), dtype=np.int16)
    perm2 = np.zeros((NCORES, NGROUP, P2), dtype=np.int16)
    nodemap = np.full((NCORES, NGROUP, P2), -1, dtype=np.int64)  # -> global node
    order = np.lexsort((gid, grp, quarter, core))
    so, go_, qo, co, gi = srcl[order], grp[order], quarter[order], core[order], gid[order]
    for c in range(NCORES):
        for g in range(NGROUP):
            qbase = 0
            for q in range(NQ):
                m = (co == c) & (go_ == g) & (qo == q)
                s_e, gi_e = so[m], gi[m]
                kv = kap[c, q, g]
                kvb = lut[kv]
                nodes = np.nonzero(kv)[0]
                kn = kv[nodes]
                knb = kvb[nodes]
                nd = np.lexsort((nodes, knb))
                nodes_s, kn_s, knb_s = nodes[nd], kn[nd], knb[nd]
                _, descr, _, kbase = layouts[q]
                rank = np.zeros(len(nodes_s), dtype=np.int64)
                colof = np.zeros(len(nodes_s), dtype=np.int64)
                for k in np.unique(knb_s):
                    mk = knb_s == k
                    rank[mk] = np.arange(mk.sum())
                    colof[mk] = kbase[int(k)]
                node_col = colof + rank
                col2slot = np.full(layouts[q][2], -1, dtype=np.int64)
                for soff, n_rows, k, col in descr:
                    cols = np.arange(n_rows)
                    col2slot[col + cols] = soff + cols * k
                starts = col2slot[node_col]
                eslots = np.repeat(starts, kn_s) + _concat_aranges(kn_s)
                ptr = np.zeros(GNN + 1, dtype=np.int64)
                ptr[1:] = np.cumsum(kv)
                ev = (
                    np.concatenate([s_e[ptr[n] : ptr[n + 1]] for n in nodes_s])
                    if len(nodes_s)
                    else np.zeros(0, dtype=np.int32)
                )
                eidx2[c, g, qbase + eslots] = ev.astype(np.int16)
                qbase += SQ2[q]
                # perm for this quarter accumulates into same node cols later;
                # here: node n (local gid) col in accp_q
                # we need per-quarter perms; store packed later
            # perms built per quarter below

    # per-quarter perms + final node mapping
    perm2q = np.zeros((NCORES, NGROUP, NQ, P2), dtype=np.int16)
    for c in range(NCORES):
        for g in range(NGROUP):
            for q in range(NQ):
                kv = kap[c, q, g]
                kvb = lut[kv]
                nodes = np.nonzero(kv)[0]
                knb = kvb[nodes]
                nd = np.lexsort((nodes, knb))
                nodes_s, knb_s = nodes[nd], knb[nd]
                _, _, _, kbase = layouts[q]
                rank = np.zeros(len(nodes_s), dtype=np.int64)
                colof = np.zeros(len(nodes_s), dtype=np.int64)
                for k in np.unique(knb_s):
                    mk = knb_s == k
                    rank[mk] = np.arange(mk.sum())
                    colof[mk] = kbase[int(k)]
                pm = np.zeros(GNN, dtype=np.int16)
                pm[nodes_s] = (colof + rank).astype(np.int16)
                perm2q[c, g, q, :GNN] = pm
            for j in range(GNN):
                n_global = (c * NSH) + (j * NGROUP + g)
                if j * NGROUP + g < NSH:
                    nodemap[c, g, j] = n_global

    # wrapped arrays
    eidx2_w = np.zeros((NCORES, P, sum(SQ2) // 16), dtype=np.int16)
    perm2_w = np.zeros((NCORES, P, NQ * (P2 // 16)), dtype=np.int16)
    for c in range(NCORES):
        for g in range(NGROUP):
            eidx2_w[c, g * 16 : (g + 1) * 16] = _wrap16(eidx2[c, g])
            perm2_w[c, g * 16 : (g + 1) * 16] = np.concatenate(
                [_wrap16(perm2q[c, g, q]) for q in range(NQ)], axis=1
            )

    dinvP = np.zeros((NCORES, NGROUP, P2), dtype=np.float32)
    for c in range(NCORES):
        for g in range(NGROUP):
            for j in range(GNN):
                n = j * NGROUP + g
                if n < NSH:
                    dinvP[c, g, j] = dinv[c * NSH + n]

    meta2 = dict(layouts=layouts, SQ2=SQ2, P2=P2, QC2=QC2, nodemap=nodemap, b2=float(b2[0]))
    k2_inputs = []
    for c in range(NCORES):
        d = {
            "eidx2": np.ascontiguousarray(eidx2_w[c]),
            "perm2": np.ascontiguousarray(perm2_w[c]),
            "dinvp": np.ascontiguousarray(dinvP[c]),
        }
        if ztab is not None:
            d["ztab"] = np.ascontiguousarray(ztab.reshape(NQ * NGROUP, QC2))
        k2_inputs.append(d)
    return k2_inputs, meta2


def build_k2(meta2):
    layouts, SQ2, P2, QC2 = meta2["layouts"], meta2["SQ2"], meta2["P2"], meta2["QC2"]
    b2 = meta2["b2"]
    nc = bacc.Bacc(None, target_bir_lowering=False)
    f32, i16 = mybir.dt.float32, mybir.dt.int16
    ztab_d = nc.dram_tensor("ztab", [NQ * NGROUP, QC2], f32, kind="ExternalInput")
    eidx_d = nc.dram_tensor("eidx2", [P, sum(SQ2) // 16], i16, kind="ExternalInput")
    perm_d = nc.dram_tensor("perm2", [P, NQ * (P2 // 16)], i16, kind="ExternalInput")
    dinvp_d = nc.dram_tensor("dinvp", [NGROUP, P2], f32, kind="ExternalInput")
    out_d = nc.dram_tensor("out2", [NGROUP, P2], f32, kind="ExternalOutput")

    with ExitStack() as ctx:
        tc = ctx.enter_context(TileContext(nc))
        pool = ctx.enter_context(tc.tile_pool(name="pool", bufs=1))
        gpool = ctx.enter_context(tc.tile_pool(name="g2", bufs=3))
        eidx = pool.tile([P, sum(SQ2) // 16], i16)
        perm = pool.tile([P, NQ * (P2 // 16)], i16)
        acc = pool.tile([P, P2], f32)
        accp = pool.tile([P, P2], f32)
        dinvp = pool.tile([P, P2], f32)
        nc.sync.dma_start(out=eidx[:], in_=eidx_d[:])
        nc.sync.dma_start(out=perm[:], in_=perm_d[:])
        nc.sync.dma_start(out=dinvp[0:NGROUP * 16:16, :], in_=dinvp_d[:])
        nc.vector.memset(accp[:, 0:1], 0.0)

        with tc.tile_pool(name="ztabs", bufs=2) as ztabs:
            sq_base = 0
            for q in range(NQ):
                slots, descr, _, _ = layouts[q]
                zt = ztabs.tile([P, QC2], f32, tag="zt")
                nc.sync.dma_start(
                    out=zt[0:NGROUP * 16:16, :], in_=ztab_d[q * NGROUP : (q + 1) * NGROUP, :]
                )
                g = gpool.tile([P, max(_pad16(max(SQ2)), 16)], f32, tag="g")
                nc.gpsimd.ap_gather(
                    g[:, :slots], zt[:], eidx[:, sq_base // 16 : (sq_base + slots) // 16],
                    channels=P, num_elems=QC2, d=1, num_idxs=slots,
                )
                for soff, n_rows, k, col in descr:
                    nc.vector.tensor_reduce(
                        accp[:, col : col + n_rows],
                        g[:, soff : soff + n_rows * k].rearrange("p (a b) -> p a b", a=n_rows, b=k),
                        axis=mybir.AxisListType.X, op=mybir.AluOpType.add,
                    )
                t = gpool.tile([P, max(_pad16(max(SQ2)), 16)], f32, tag="g")
                nc.gpsimd.ap_gather(
                    t[:, :P2], accp[:], perm[:, q * (P2 // 16) : (q + 1) * (P2 // 16)],
                    channels=P, num_elems=P2, d=1, num_idxs=P2,
                )
                if q == 0:
                    nc.vector.tensor_copy(acc[:], t[:, :P2])
                else:
                    nc.vector.tensor_add(acc[:], acc[:], t[:, :P2])
                sq_base += slots

        nc.vector.tensor_mul(acc[:], acc[:], dinvp[:])
        nc.scalar.activation(acc[:], acc[:], mybir.ActivationFunctionType.Sigmoid, bias=b2)
        nc.sync.dma_start(out=out_d[:], in_=acc[0:NGROUP * 16:16, :])
    nc.finalize()
    return nc


def _sim_ns(nc):
    from concourse import bass_interp

    sim = bass_interp.CoreSim(nc, no_exec=True, publish_trace=False)
    sim.simulate()
    return int(sim.time)


def build_fused(meta, meta2):
    """Single-launch: layer 1 + on-device AllGather of z' + layer 2."""
    layouts, SQ, PQ, PERM_NI = meta["layouts"], meta["SQ"], meta["PQ"], meta["PERM_NI"]
    layouts2, SQ2, P2, QC2 = meta2["layouts"], meta2["SQ2"], meta2["P2"], meta2["QC2"]
    b2 = meta2["b2"]
    nc = bacc.Bacc(None, target_bir_lowering=False)
    f32, i16 = mybir.dt.float32, mybir.dt.int16
    xt_d = nc.dram_tensor("xt", [P, NQ * QCOLS], f32, kind="ExternalInput")
    w1_d = nc.dram_tensor("w1", [P, P], f32, kind="ExternalInput")
    b1_d = nc.dram_tensor("b1", [P, 1], f32, kind="ExternalInput")
    w2_d = nc.dram_tensor("w2", [P, 1], f32, kind="ExternalInput")
    eidx_d = nc.dram_tensor("eidx", [P, sum(SQ) // 16], i16, kind="ExternalInput")
    perm_d = nc.dram_tensor("perm", [P, NQ * (PERM_NI // 16)], i16, kind="ExternalInput")
    dinvb_d = nc.dram_tensor("dinvb", [P, NSH], f32, kind="ExternalInput")
    eidx2_d = nc.dram_tensor("eidx2", [P, sum(SQ2) // 16], i16, kind="ExternalInput")
    perm2_d = nc.dram_tensor("perm2", [P, NQ * (P2 // 16)], i16, kind="ExternalInput")
    dinvp_d = nc.dram_tensor("dinvp", [NGROUP, P2], f32, kind="ExternalInput")
    out_d = nc.dram_tensor("out2", [NGROUP, P2], f32, kind="ExternalOutput")

    with ExitStack() as ctx:
        tc = ctx.enter_context(TileContext(nc))
        cpool = ctx.enter_context(tc.tile_pool(name="cpool", bufs=1))
        dram = ctx.enter_context(tc.tile_pool(name="dram", bufs=1, space="DRAM"))
        w1 = cpool.tile([P, P], f32)
        b1 = cpool.tile([P, 1], f32)
        w2 = cpool.tile([P, 1], f32)
        eidx = cpool.tile([P, sum(SQ) // 16], i16)
        perm = cpool.tile([P, NQ * (PERM_NI // 16)], i16)
        zin = nc.dram_tensor("zin_cc", [NGROUP, NSH], f32, kind="Internal")
        zall = nc.dram_tensor("zall_cc", [NGROUP * NCORES, NSH], f32, kind="Internal", addr_space="Shared")
        nc.sync.dma_start(out=w1[:], in_=w1_d[:])
        nc.sync.dma_start(out=b1[:], in_=b1_d[:])
        nc.sync.dma_start(out=w2[:], in_=w2_d[:])
        nc.sync.dma_start(out=eidx[:], in_=eidx_d[:])
        nc.sync.dma_start(out=perm[:], in_=perm_d[:])

        with tc.tile_pool(name="apool", bufs=1) as apool:
            acc = apool.tile([P, NSH], f32)
            accp = apool.tile([P, PQ], f32)
            nc.vector.memset(accp[:, 0:1], 0.0)
            with (
                tc.tile_pool(name="tabs", bufs=2) as tabs,
                tc.tile_pool(name="xpool", bufs=2) as xpool,
                tc.tile_pool(name="gpool", bufs=2) as gpool,
                tc.tile_pool(name="pspool", bufs=2, space="PSUM") as pspool,
            ):
                sq_base = 0
                for q in range(NQ):
                    n_chunks, descr, _, _ = layouts[q]
                    tab = tabs.tile([P, QCOLS], f32, tag="tab")
                    XB = 2 * MMCH
                    for x0 in range(0, QCOLS, XB):
                        xw = min(XB, QCOLS - x0)
                        xc = xpool.tile([P, XB], f32, tag="x")
                        nc.sync.dma_start(
                            out=xc[:, :xw], in_=xt_d[:, q * QCOLS + x0 : q * QCOLS + x0 + xw]
                        )
                        for m0 in range(0, xw, MMCH):
                            ps = pspool.tile([P, MMCH], f32, tag="ps")
                            nc.tensor.matmul(ps[:], w1[:], xc[:, m0 : m0 + MMCH], start=True, stop=True)
                            nc.scalar.activation(
                                tab[:, x0 + m0 : x0 + m0 + MMCH], ps[:],
                                mybir.ActivationFunctionType.Copy,
                            )
                    by_chunk = {}
                    for d_ in descr:
                        by_chunk.setdefault(d_[0], []).append(d_)
                    for ch in range(n_chunks):
                        g = gpool.tile([P, G1], f32, tag="g")
                        i0 = (sq_base + ch * G1) // 16
                        nc.gpsimd.ap_gather(
                            g[:], tab[:], eidx[:, i0 : i0 + G1 // 16],
                            channels=P, num_elems=QCOLS, d=1, num_idxs=G1,
                        )
                        for (_, off, n_rows, k, col) in by_chunk.get(ch, []):
                            nc.vector.tensor_reduce(
                                accp[:, col : col + n_rows],
                                g[:, off : off + n_rows * k].rearrange(
                                    "p (a b) -> p a b", a=n_rows, b=k
                                ),
                                axis=mybir.AxisListType.X, op=mybir.AluOpType.add,
                            )
                    pbase = q * (PERM_NI // 16)
                    for s0 in range(0, PERM_NI, G1):
                        w = min(G1, PERM_NI - s0)
                        w = min(w, NSH - s0) if s0 < NSH else 0
                        if w <= 0:
                            break
                        wp = _pad16(w)
                        t = gpool.tile([P, G1], f32, tag="g")
                        nc.gpsimd.ap_gather(
                            t[:, :wp], accp[:], perm[:, pbase + s0 // 16 : pbase + (s0 + wp) // 16],
                            channels=P, num_elems=PQ, d=1, num_idxs=wp,
                        )
                        if q == 0:
                            nc.scalar.activation(
                                acc[:, s0 : s0 + w], t[:, :w], mybir.ActivationFunctionType.Copy
                            )
                        else:
                            nc.vector.tensor_add(acc[:, s0 : s0 + w], acc[:, s0 : s0 + w], t[:, :w])
                    sq_base += SQ[q]

            with (
                tc.tile_pool(name="fin", bufs=1) as fin,
                tc.tile_pool(name="zpspool", bufs=2, space="PSUM") as zps,
            ):
                dinvb = fin.tile([P, NSH], f32)
                zrow = fin.tile([1, NSH], f32)
                nc.sync.dma_start(out=dinvb[:], in_=dinvb_d[:])
                for f0 in range(0, NSH, G1):
                    fw = min(G1, NSH - f0)
                    sl = slice(f0, f0 + fw)
                    nc.vector.tensor_mul(acc[:, sl], acc[:, sl], dinvb[:, sl])
                    nc.scalar.activation(
                        acc[:, sl], acc[:, sl], mybir.ActivationFunctionType.Sigmoid, bias=b1[:, 0:1]
                    )
                    nc.vector.tensor_mul(acc[:, sl], acc[:, sl], dinvb[:, sl])
                for m0 in range(0, NSH, MMCH):
                    w = min(MMCH, NSH - m0)
                    ps = zps.tile([1, MMCH], f32, tag="zps")
                    nc.tensor.matmul(ps[:, :w], w2[:], acc[:, m0 : m0 + w], start=True, stop=True)
                    nc.scalar.activation(zrow[:, m0 : m0 + w], ps[:, :w], mybir.ActivationFunctionType.Copy)
                for g_ in range(NGROUP):
                    nc.sync.dma_start(out=zin[g_ : g_ + 1, :], in_=zrow[:])

        nc.gpsimd.collective_compute(
            "AllGather", mybir.AluOpType.bypass,
            replica_groups=[list(range(NCORES))],
            ins=[zin[:].opt()], outs=[zall[:].opt()],
        )

        # ---- layer 2 ----
        with (
            tc.tile_pool(name="k2pool", bufs=1) as pool2,
            tc.tile_pool(name="ztabs", bufs=2) as ztabs,
            tc.tile_pool(name="g2", bufs=3) as gpool2,
        ):
            eidx2 = pool2.tile([P, sum(SQ2) // 16], i16)
            perm2 = pool2.tile([P, NQ * (P2 // 16)], i16)
            acc2 = pool2.tile([P, P2], f32)
            accp2 = pool2.tile([P, P2], f32)
            dinvp = pool2.tile([P, P2], f32)
            nc.sync.dma_start(out=eidx2[:], in_=eidx2_d[:])
            nc.sync.dma_start(out=perm2[:], in_=perm2_d[:])
            nc.sync.dma_start(out=dinvp[0 : NGROUP * 16 : 16, :], in_=dinvp_d[:])
            nc.vector.memset(accp2[:, 0:1], 0.0)
            GSZ = max(_pad16(max(SQ2)), P2, 16)
            half = NSH
            sq_base = 0
            for q in range(NQ):
                slots, descr, _, _ = layouts2[q]
                zt = ztabs.tile([P, QC2], f32, tag="zt")
                nc.vector.memset(zt[:, 0:1], 0.0)
                nc.sync.dma_start(
                    out=zt[0 : NGROUP * 16 : 16, 1 : 1 + half],
                    in_=zall[NGROUP * (2 * q) : NGROUP * (2 * q) + NGROUP, :],
                )
                nc.sync.dma_start(
                    out=zt[0 : NGROUP * 16 : 16, 1 + half : 1 + 2 * half],
                    in_=zall[NGROUP * (2 * q + 1) : NGROUP * (2 * q + 1) + NGROUP, :],
                )
                g = gpool2.tile([P, GSZ], f32, tag="g")
                nc.gpsimd.ap_gather(
                    g[:, :slots], zt[:], eidx2[:, sq_base // 16 : (sq_base + slots) // 16],
                    channels=P, num_elems=QC2, d=1, num_idxs=slots,
                )
                for soff, n_rows, k, col in descr:
                    nc.vector.tensor_reduce(
                        accp2[:, col : col + n_rows],
                        g[:, soff : soff + n_rows * k].rearrange("p (a b) -> p a b", a=n_rows, b=k),
                        axis=mybir.AxisListType.X, op=mybir.AluOpType.add,
                    )
                t = gpool2.tile([P, GSZ], f32, tag="g")
                nc.gpsimd.ap_gather(
                    t[:, :P2], accp2[:], perm2[:, q * (P2 // 16) : (q + 1) * (P2 // 16)],
                    channels=P, num_elems=P2, d=1, num_idxs=P2,
                )
                if q == 0:
                    nc.scalar.activation(acc2[:], t[:, :P2], mybir.ActivationFunctionType.Copy)
                else:
                    nc.vector.tensor_add(acc2[:], acc2[:], t[:, :P2])
                sq_base += slots

            nc.vector.tensor_mul(acc2[:], acc2[:], dinvp[:])
            nc.scalar.activation(acc2[:], acc2[:], mybir.ActivationFunctionType.Sigmoid, bias=b2)
            nc.sync.dma_start(out=out_d[:], in_=acc2[0 : NGROUP * 16 : 16, :])
    nc.finalize()
    return nc


def _assemble_out(results, meta2):
    out = np.zeros((N, 1), dtype=np.float32)
    nodemap = meta2["nodemap"]
    for c in range(NCORES):
        o = results[c]["out2"]  # [8, P2]
        valid = nodemap[c] >= 0
        out[nodemap[c][valid], 0] = o[valid]
    return out


def kernel(x, edge_index, W1, b1, W2, b2):
    global LAST_SIM_NS
    x = np.asarray(x, dtype=np.float32)
    edge_index = np.asarray(edge_index)
    k1_inputs, meta, (src, dst, dinv) = host_prep(x, edge_index, W1, b1, W2, b2)
    b2np = np.asarray(b2, dtype=np.float32)
    try:
        # single launch: layer1 + AllGather(z') + layer2 fused in one NEFF
        k2_inputs, meta2 = host_prep_k2(None, src, dst, dinv, b2np)
        nc = build_fused(meta, meta2)
        if MEASURE:
            LAST_SIM_NS = _sim_ns(nc)
        in_maps = [dict(k1_inputs[c], **k2_inputs[c]) for c in range(NCORES)]
        res = run_bass_kernel_spmd(nc, in_maps, list(range(NCORES)))
        return _assemble_out(res.results, meta2)
    except Exception:
        import traceback

        traceback.print_exc()

    # fallback: two launches with z' crossing via host
    nc1 = build_k1(meta)
    sim1 = _sim_ns(nc1) if MEASURE else 0
    res1 = run_bass_kernel_spmd(nc1, k1_inputs, list(range(NCORES)))
    zfull = np.concatenate([res1.results[c]["zout"][0, :NSH] for c in range(NCORES)])
    k2_inputs, meta2 = host_prep_k2(zfull, src, dst, dinv, b2np)
    nc2 = build_k2(meta2)
    if MEASURE:
        LAST_SIM_NS = sim1 + _sim_ns(nc2)
    res2 = run_bass_kernel_spmd(nc2, k2_inputs, list(range(NCORES)))
    return _assemble_out(res2.results, meta2)



# revision 2
# speedup vs baseline: 1.0231x; 1.0231x over previous
"""2-layer GCN (PyG GCNConv x2 + sigmoid) on 8 TRN2 NeuronCores, single fused NEFF.

v2 design (cost-model-driven rewrite of the quarter-table baseline):
- ap_gather costs max(table_width, num_idxs) * 0.833ns on GPSIMD. The baseline
  gathered 2560 idxs from 12800-wide tables (5x overpay). Here: 8 octant
  tables of 6416 cols, gathered in 2 chunks of ~SQ_o/2 >= 6416 idxs each ->
  ~0.833ns/slot, GPSIMD edge-gather cost drops ~5x.
- Table build matmuls in bf16 (1 PE cycle/row vs 4 for fp32); x is uploaded
  pre-scaled by dinv[src] as bf16 (halves DMA too).
- Octant assembly: per-octant perm-gather to dst order, then nodes 0..3071
  accumulate over octants in PSUM via identity-matmul (PE), nodes 3072..6249
  via tensor_add split between GPSIMD and DVE (PSUM can't hold all 6250 f32).
- Layer 2 is src-grouped: the 8 gpsimd 16-partition groups hold (src-quarter,
  dst-half) z tables; one gather covers all ~15k layer-2 edge slots at
  num_idxs > table width; cross-src-quarter combine happens on the PE with a
  0/1 stationary matrix into a [2, 3136] output.
- z' = dinv_dst * (W2^T sigma1) AllGathered as one 25KB row per core
  (15us collective constant + ~5us payload).
"""

import sys

sys.path.insert(0, "/opt/trn_rl_repo")
import numpy as np
from contextlib import ExitStack

from concourse import bacc, mybir
from concourse.tile import TileContext
from concourse.bass_utils import run_bass_kernel_spmd

try:
    import ml_dtypes

    _BF16 = np.dtype(ml_dtypes.bfloat16)
except Exception:  # pragma: no cover
    _BF16 = None

MEASURE = False
LAST_SIM_NS = None

N = 50000
E = 800000
F = 128
P = 128
NCORES = 8
NSH = N // NCORES  # 6250 dst nodes per core
NO = 8  # src octants (tables)
ON = N // NO  # 6250 src nodes per octant
OCOLS = 6256  # octant table cols: [zero, 6250 nodes, pad] mult of 16
PERM_NI = 6256  # pad16(NSH): assembly perm idx count per octant
H0 = 3072  # nodes 0..H0-1 accumulate in PSUM via PE
H1 = PERM_NI - H0  # 3200 (covers nodes H0..6249 + pad)
GMAX = 7808  # max L1 gather chunk (slots)
MMCH = 512  # matmul moving chunk
K2G = 8  # layer-2 partition groups = (src quarter, dst half)
DH = NSH // 2  # 3125 dst nodes per k2 half
DHP = 3136  # pad16(DH)
K2W = 4 * DHP + 16  # k2 table cols: [zero, 4 half-rows of 3136, pad]


def _wrap16(idx_flat):
    n = idx_flat.shape[0]
    assert n % 16 == 0
    return np.ascontiguousarray(idx_flat.reshape(n // 16, 16).T)


def _pad16(n, mult=16):
    return ((n + mult - 1) // mult) * mult


def _concat_aranges(lens):
    if len(lens) == 0:
        return np.zeros(0, dtype=np.int64)
    total = int(lens.sum())
    out = np.ones(total, dtype=np.int64)
    ends = np.cumsum(lens)
    out[0] = 0
    out[ends[:-1]] = -(lens[:-1] - 1)
    return np.cumsum(out)


def _bucket_lut(kmax, exact, buckets):
    lut = np.arange(max(kmax + 1, exact + 1))
    for kk in range(exact + 1, len(lut)):
        for bb in buckets:
            if kk <= bb:
                lut[kk] = bb
                break
        else:
            lut[kk] = ((kk + 63) // 64) * 64
    return lut


def _ladder_layout(kapb_all, cap_fn):
    """kapb_all: [NCORES, NCELLS, NNODES] bucketed kappas sharing one layout.
    Returns (budgets {k: n}, chunks list, descr [(ch, off, n_rows, k, col)],
    n_cols)."""
    b = {}
    for k in np.unique(kapb_all):
        k = int(k)
        if k == 0:
            continue
        nk = int((kapb_all == k).sum(axis=-1).max())
        if nk > 0:
            b[k] = nk
    raw = sum(k * n for k, n in b.items())
    cap = cap_fn(raw)
    descr = []
    col = 1
    ch, off = 0, 0
    for k in sorted(b):
        left = b[k]
        while left > 0:
            fit = min(left, (cap - off) // k)
            if fit == 0:
                ch += 1
                off = 0
                fit = min(left, cap // k)
            descr.append((ch, off, fit, k, col))
            off += fit * k
            col += fit
            left -= fit
    n_chunks = ch + 1
    return b, n_chunks, cap, descr, col


def _fill_slots(kv, lut, descr, col2k_base, s_sorted, cap):
    """Place each node's edges into its ladder row. kv: per-node actual count;
    s_sorted: edge values sorted by node. Returns (slot_positions, values,
    node_cols)."""
    kvb = lut[kv]
    nodes = np.nonzero(kv)[0]
    kn = kv[nodes]
    knb = kvb[nodes]
    nd = np.lexsort((nodes, knb))
    nodes_s, kn_s, knb_s = nodes[nd], kn[nd], knb[nd]
    rank = np.zeros(len(nodes_s), dtype=np.int64)
    colof = np.zeros(len(nodes_s), dtype=np.int64)
    for k in np.unique(knb_s):
        mk = knb_s == k
        rank[mk] = np.arange(mk.sum())
        colof[mk] = col2k_base[int(k)]
    node_col = colof + rank
    ncols = max(d[4] + d[2] for d in descr)
    col2slot = np.full(ncols, -1, dtype=np.int64)
    for ch, off, n_rows, k, col in descr:
        cols = np.arange(n_rows)
        col2slot[col + cols] = ch * cap + off + cols * k
    starts = col2slot[node_col]
    eslots = np.repeat(starts, kn_s) + _concat_aranges(kn_s)
    ptr = np.zeros(len(kv) + 1, dtype=np.int64)
    ptr[1:] = np.cumsum(kv)
    ev = (
        np.concatenate([s_sorted[ptr[n] : ptr[n + 1]] for n in nodes_s])
        if len(nodes_s)
        else np.zeros(0, dtype=np.int64)
    )
    pm = np.zeros(len(kv), dtype=np.int16)
    pm[nodes_s] = node_col.astype(np.int16)
    return eslots, ev, pm


def host_prep(x, edge_index, W1, b1, W2, b2):
    src = np.concatenate([edge_index[0], np.arange(N, dtype=np.int64)]).astype(np.int32)
    dst = np.concatenate([edge_index[1], np.arange(N, dtype=np.int64)]).astype(np.int32)
    deg = np.bincount(dst, minlength=N).astype(np.float32)
    dinv = 1.0 / np.sqrt(np.maximum(deg, 1e-12))
    dinv[deg <= 0] = 0.0

    # random node->table-position permutation balances per-(core,octant)
    # degree distributions, keeping shared max-over-core budgets tight
    psrc = np.random.default_rng(12345).permutation(N)
    pinv = np.argsort(psrc)

    xtp = (x * dinv[:, None]).T.astype(np.float32)[:, pinv]  # [128, N] pos order
    xt = np.zeros((P, NO * OCOLS), dtype=np.float32)
    for o in range(NO):
        xt[:, o * OCOLS + 1 : o * OCOLS + 1 + ON] = xtp[:, o * ON : (o + 1) * ON]
    xt_bf16 = xt.astype(_BF16)

    core = dst // NSH
    dstl = dst % NSH
    pos = psrc[src]
    octant = pos // ON
    srcl = (pos % ON).astype(np.int64) + 1

    # kappa per (core, octant, local dst node)
    kap = np.zeros((NCORES, NO, NSH), dtype=np.int32)
    for c in range(NCORES):
        mc = core == c
        for o in range(NO):
            m = mc & (octant == o)
            kap[c, o] = np.bincount(dstl[m], minlength=NSH)

    kmax = int(kap.max())
    lut = _bucket_lut(kmax, 12, (14, 16, 19, 22, 26, 32, 40, 48, 64, 96, 128, 192, 256))
    kapb = lut[kap]

    layouts = []  # per octant: (n_chunks, cap, descr, n_cols, kbase)
    for o in range(NO):
        b, n_chunks, cap, descr, ncol = _ladder_layout(
            kapb[:, o, :], lambda raw: min(GMAX, _pad16((raw + 1) // 2 + 64))
        )
        kbase = {}
        for ch, off, n_rows, k, col in descr:
            kbase.setdefault(k, col)
        layouts.append((n_chunks, cap, descr, ncol, kbase))

    SQ = [layouts[o][0] * layouts[o][1] for o in range(NO)]  # slots per octant
    PQ = _pad16(max(layouts[o][3] for o in range(NO)))

    order = np.lexsort((dstl, octant, core))
    so, do_, oo, co = srcl[order], dstl[order], octant[order], core[order]
    eidx = np.zeros((NCORES, sum(SQ)), dtype=np.int16)
    perms = np.zeros((NCORES, NO, PERM_NI), dtype=np.int16)
    for c in range(NCORES):
        obase = 0
        for o in range(NO):
            m = (co == c) & (oo == o)
            _, cap, descr, _, kbase = layouts[o]
            eslots, ev, pm = _fill_slots(kap[c, o], lut, descr, kbase, so[m], cap)
            eidx[c, obase + eslots] = ev.astype(np.int16)
            perms[c, o, :NSH] = pm
            obase += SQ[o]

    eidx_w = np.zeros((NCORES, P, sum(SQ) // 16), dtype=np.int16)
    perm_w = np.zeros((NCORES, P, NO * (PERM_NI // 16)), dtype=np.int16)
    for c in range(NCORES):
        eidx_w[c] = np.tile(_wrap16(eidx[c]), (K2G, 1))
        pw = np.concatenate([_wrap16(perms[c, o]) for o in range(NO)], axis=1)
        perm_w[c] = np.tile(pw, (K2G, 1))

    # ---------------- layer 2 (src-grouped) ----------------
    # Appended self-loops are excluded (their z'[n] term is added on-device
    # from the local z row); only the original E edges go through the gather.
    # group g = 2*src_quarter + dst_half; table per group: z' of src quarter
    # laid out as 4 half-rows of DHP (cores 2sq,2sq+1 x dst-halves), matching
    # the AllGather result zall [16, DHP].
    src2 = src[:E]
    dst2 = dst[:E]
    core2 = dst2 // NSH
    dstl2 = dst2 % NSH
    srcq = (src2 // (2 * NSH)).astype(np.int64)  # 0..3
    _c2 = (src2 // NSH).astype(np.int64) % 2  # core parity within quarter
    _i2 = (src2 % NSH).astype(np.int64)
    srcl2 = (2 * _c2 + _i2 // DH) * DHP + (_i2 % DH)  # table position (0-based)
    dh = dstl2 // DH  # 0..1
    j2 = dstl2 % DH  # 0..3124
    grp = 2 * srcq + dh

    kap2 = np.zeros((NCORES, K2G, DH), dtype=np.int32)
    for c in range(NCORES):
        mc = core2 == c
        for g in range(K2G):
            m = mc & (grp == g)
            kap2[c, g] = np.bincount(j2[m], minlength=DH)

    kmax2 = int(kap2.max())
    lut2 = _bucket_lut(kmax2, 9, (11, 13, 15, 18, 22, 27, 33, 40, 48, 64, 96, 128, 192, 256))
    kapb2 = lut2[kap2]
    b2_, n_chunks2, cap2, descr2, ncol2 = _ladder_layout(
        kapb2.reshape(NCORES, K2G, DH), lambda raw: _pad16(raw)
    )
    assert n_chunks2 == 1
    SQ2 = cap2
    P2 = _pad16(ncol2)
    kbase2 = {}
    for ch, off, n_rows, k, col in descr2:
        kbase2.setdefault(k, col)

    order2 = np.lexsort((j2, grp, core2))
    so2, jo2, go2, co2 = srcl2[order2], j2[order2], grp[order2], core2[order2]
    eidx2 = np.full((NCORES, K2G, SQ2), 4 * DHP, dtype=np.int16)  # pad -> zero col
    perm2 = np.zeros((NCORES, K2G, DHP), dtype=np.int16)
    for c in range(NCORES):
        for g in range(K2G):
            m = (co2 == c) & (go2 == g)
            eslots, ev, pm = _fill_slots(kap2[c, g], lut2, descr2, kbase2, so2[m], cap2)
            eidx2[c, g, eslots] = ev.astype(np.int16)
            perm2[c, g, :DH] = pm

    eidx2_w = np.zeros((NCORES, P, SQ2 // 16), dtype=np.int16)
    perm2_w = np.zeros((NCORES, P, DHP // 16), dtype=np.int16)
    for c in range(NCORES):
        for g in range(K2G):
            eidx2_w[c, g * 16 : (g + 1) * 16] = _wrap16(eidx2[c, g])
            perm2_w[c, g * 16 : (g + 1) * 16] = _wrap16(perm2[c, g])

    ident = np.eye(P, dtype=np.float32)
    wones = np.zeros((P, 2), dtype=np.float32)
    for g in range(K2G):
        wones[16 * g, g % 2] = 1.0

    dinvb = np.zeros((NCORES, P, NSH), dtype=np.float32)
    dinvd2 = np.zeros((NCORES, 2, DHP), dtype=np.float32)
    for c in range(NCORES):
        dsh = dinv[c * NSH : (c + 1) * NSH]
        dinvb[c] = np.tile(dsh, (P, 1))
        dinvd2[c, 0, :DH] = dsh[:DH]
        dinvd2[c, 1, :DH] = dsh[DH:]

    meta = dict(layouts=layouts, SQ=SQ, PQ=PQ, descr2=descr2, SQ2=SQ2, P2=P2,
                b2=float(np.asarray(b2).reshape(-1)[0]))
    inputs = []
    for c in range(NCORES):
        inputs.append(
            {
                "xt": xt_bf16,
                "w1": W1.astype(np.float32).astype(_BF16),
                "b1": np.asarray(b1, dtype=np.float32).reshape(P, 1),
                "w2": np.asarray(W2, dtype=np.float32).reshape(P, 1),
                "ident": ident,
                "wones": wones,
                "eidx": np.ascontiguousarray(eidx_w[c]),
                "perm": np.ascontiguousarray(perm_w[c]),
                "eidx2": np.ascontiguousarray(eidx2_w[c]),
                "perm2": np.ascontiguousarray(perm2_w[c]),
                "dinvb": np.ascontiguousarray(dinvb[c]),
                "dinvd2": np.ascontiguousarray(dinvd2[c]),
            }
        )
    return inputs, meta


def build_fused(meta):
    layouts, SQ, PQ = meta["layouts"], meta["SQ"], meta["PQ"]
    descr2, SQ2, P2 = meta["descr2"], meta["SQ2"], meta["P2"]
    b2 = meta["b2"]
    nc = bacc.Bacc(None, target_bir_lowering=False)
    f32, bf16, i16 = mybir.dt.float32, mybir.dt.bfloat16, mybir.dt.int16

    xt_d = nc.dram_tensor("xt", [P, NO * OCOLS], bf16, kind="ExternalInput")
    w1_d = nc.dram_tensor("w1", [P, P], bf16, kind="ExternalInput")
    b1_d = nc.dram_tensor("b1", [P, 1], f32, kind="ExternalInput")
    w2_d = nc.dram_tensor("w2", [P, 1], f32, kind="ExternalInput")
    ident_d = nc.dram_tensor("ident", [P, P], f32, kind="ExternalInput")
    wones_d = nc.dram_tensor("wones", [P, 2], f32, kind="ExternalInput")
    eidx_d = nc.dram_tensor("eidx", [P, sum(SQ) // 16], i16, kind="ExternalInput")
    perm_d = nc.dram_tensor("perm", [P, NO * (PERM_NI // 16)], i16, kind="ExternalInput")
    eidx2_d = nc.dram_tensor("eidx2", [P, SQ2 // 16], i16, kind="ExternalInput")
    perm2_d = nc.dram_tensor("perm2", [P, DHP // 16], i16, kind="ExternalInput")
    dinvb_d = nc.dram_tensor("dinvb", [P, NSH], f32, kind="ExternalInput")
    dinvd2_d = nc.dram_tensor("dinvd2", [2, DHP], f32, kind="ExternalInput")
    out_d = nc.dram_tensor("out2", [2, DHP], f32, kind="ExternalOutput")
    zin = nc.dram_tensor("zin_cc", [2, DHP], bf16, kind="Internal")
    zall = nc.dram_tensor("zall_cc", [4, 4 * DHP], bf16, kind="Internal", addr_space="Shared")

    with ExitStack() as ctx:
        tc = ctx.enter_context(TileContext(nc))
        cpool = ctx.enter_context(tc.tile_pool(name="cpool", bufs=1))
        w1 = cpool.tile([P, P], bf16)
        b1 = cpool.tile([P, 1], f32)
        w2 = cpool.tile([P, 1], f32)
        ident = cpool.tile([P, P], f32)
        wones = cpool.tile([P, 2], f32)
        perm = cpool.tile([P, NO * (PERM_NI // 16)], i16)
        nc.scalar.dma_start(out=w1[:], in_=w1_d[:])
        nc.scalar.dma_start(out=b1[:], in_=b1_d[:])
        nc.scalar.dma_start(out=w2[:], in_=w2_d[:])
        nc.scalar.dma_start(out=ident[:], in_=ident_d[:])
        nc.scalar.dma_start(out=wones[:], in_=wones_d[:])
        nc.scalar.dma_start(out=perm[:], in_=perm_d[:])


        with (
            tc.tile_pool(name="apool", bufs=1) as apool,
            tc.tile_pool(name="pshpool", bufs=1, space="PSUM") as pshpool,
        ):
            acc1 = apool.tile([P, H1], f32)
            ps_h0 = pshpool.tile([P, H0], f32)

            with (
                tc.tile_pool(name="tabs", bufs=2) as tabs,
                tc.tile_pool(name="xpool", bufs=2) as xpool,
                tc.tile_pool(name="epool", bufs=2) as epool,
                tc.tile_pool(name="gpool", bufs=2) as gpool,
                tc.tile_pool(name="appool", bufs=2) as appool,
                tc.tile_pool(name="tpool", bufs=1) as tpool,
                tc.tile_pool(name="pstab", bufs=1, space="PSUM") as pstab,
            ):
                sq_base = 0
                SQMAX = max(SQ)
                CAPMAX = max(layouts[o][1] for o in range(NO))
                prev_accp = None  # assembly for octant o runs after gathers o+1

                def _assemble(o, accp_o):
                    # perm-gather accp -> dst order, then accumulate: nodes
                    # [0,H0) into PSUM via identity matmul, [H0,) via DVE adds
                    if o == NO - 1:
                        # keep PE warm across the reduce+perm wait so the
                        # final assembly matmuls run at full clock
                        pst = pstab.tile([P, 2 * MMCH], f32, tag="pstab")
                        for i in range(5):
                            nc.tensor.matmul(
                                pst[0:1, :MMCH], w2[:],
                                accp_o[:, (i % 8) * MMCH : (i % 8 + 1) * MMCH],
                                start=True, stop=True,
                            )
                    t = tpool.tile([P, PERM_NI], f32, tag="t")
                    pbase = o * (PERM_NI // 16)
                    nc.gpsimd.ap_gather(
                        t[:], accp_o[:], perm[:, pbase : pbase + PERM_NI // 16],
                        channels=P, num_elems=PQ, d=1, num_idxs=PERM_NI,
                    )
                    for m0 in range(0, H0, MMCH):
                        nc.tensor.matmul(
                            ps_h0[:, m0 : m0 + MMCH], ident[:], t[:, m0 : m0 + MMCH],
                            start=(o == 0), stop=(o == NO - 1), skip_group_check=True,
                        )
                    if o == 0:
                        nc.scalar.activation(
                            acc1[:], t[:, H0:], mybir.ActivationFunctionType.Copy
                        )
                    else:
                        nc.vector.tensor_add(acc1[:], acc1[:], t[:, H0:])

                for o in range(NO):
                    n_chunks, cap, descr, _, _ = layouts[o]
                    tab = tabs.tile([P, OCOLS], f32, tag="tab")
                    XB = 1568 if o == 0 else 784
                    for x0 in range(0, OCOLS, XB):
                        xw = min(XB, OCOLS - x0)
                        xc = xpool.tile([P, 1568 if o == 0 else 784], bf16, tag="x")
                        nc.sync.dma_start(
                            out=xc[:, :xw], in_=xt_d[:, o * OCOLS + x0 : o * OCOLS + x0 + xw]
                        )
                        for m0 in range(0, xw, 2 * MMCH):
                            mw = min(2 * MMCH, xw - m0)
                            ps = pstab.tile([P, 2 * MMCH], f32, tag="pstab")
                            for h in range(0, mw, MMCH):
                                hw_ = min(MMCH, mw - h)
                                nc.tensor.matmul(
                                    ps[:, h : h + hw_], w1[:], xc[:, m0 + h : m0 + h + hw_],
                                    start=True, stop=True,
                                )
                            nc.scalar.activation(
                                tab[:, x0 + m0 : x0 + m0 + mw], ps[:, :mw],
                                mybir.ActivationFunctionType.Copy,
                            )
                    # gather + ladder reduce (k=1 buckets are copies -> Act)
                    eb = epool.tile([P, SQMAX // 16], i16, tag="eidx")
                    nc.sync.dma_start(
                        out=eb[:, : SQ[o] // 16],
                        in_=eidx_d[:, sq_base // 16 : (sq_base + SQ[o]) // 16],
                    )
                    accp = appool.tile([P, PQ], f32, tag="accp")
                    nc.vector.memset(accp[:, 0:1], 0.0)
                    by_chunk = {}
                    for d_ in descr:
                        by_chunk.setdefault(d_[0], []).append(d_)
                    for ch in range(n_chunks):
                        g = gpool.tile([P, CAPMAX], f32, tag="g")
                        nc.gpsimd.ap_gather(
                            g[:, :cap], tab[:], eb[:, ch * cap // 16 : (ch + 1) * cap // 16],
                            channels=P, num_elems=OCOLS, d=1, num_idxs=cap,
                        )
                        for (_, off, n_rows, k, col) in by_chunk.get(ch, []):
                            if k == 1:
                                nc.scalar.activation(
                                    accp[:, col : col + n_rows], g[:, off : off + n_rows],
                                    mybir.ActivationFunctionType.Copy,
                                )
                            else:
                                nc.vector.tensor_reduce(
                                    accp[:, col : col + n_rows],
                                    g[:, off : off + n_rows * k].rearrange(
                                        "p (a b) -> p a b", a=n_rows, b=k
                                    ),
                                    axis=mybir.AxisListType.X, op=mybir.AluOpType.add,
                                )
                    if prev_accp is not None:
                        _assemble(o - 1, prev_accp)
                    prev_accp = accp
                    sq_base += SQ[o]
                _assemble(NO - 1, prev_accp)

            # finalize layer 1: sigma1 = sigmoid(dinv*acc + b1); z' = dinv * W2^T sigma1
            # z' is kept as two 1-partition rows (dst halves at DH=3125, padded
            # to DHP), scaled by dinvd2 on write, then DMA'd to zin [2, DHP]
            with (
                tc.tile_pool(name="fin", bufs=1) as fin,
                tc.tile_pool(name="psz", bufs=2, space="PSUM") as psz,
            ):
                s0 = fin.tile([P, H0], f32)
                s1 = fin.tile([P, NSH - H0], f32)
                dinvb = fin.tile([P, NSH], f32)
                zr0 = fin.tile([1, DHP], bf16)
                zr1 = fin.tile([1, DHP], bf16)
                d20 = fin.tile([1, DHP], f32)
                d21 = fin.tile([1, DHP], f32)
                nc.scalar.dma_start(out=dinvb[:, H0:], in_=dinvb_d[:, H0:])
                nc.sync.dma_start(out=dinvb[:, :H0], in_=dinvb_d[:, :H0])
                nc.sync.dma_start(out=d20[:], in_=dinvd2_d[0:1, :])
                nc.sync.dma_start(out=d21[:], in_=dinvd2_d[1:2, :])
                nc.vector.memset(zr0[:, DH:], 0.0)
                nc.vector.memset(zr1[:, DH:], 0.0)

                def _zr_write(ps, glob0, w):
                    # ps[0, :w] holds z~ for global nodes [glob0, glob0+w)
                    done = 0
                    while done < w:
                        g0 = glob0 + done
                        dh_ = g0 // DH
                        j0 = g0 - dh_ * DH
                        n = min(w - done, DH - j0)
                        zrt = zr1 if dh_ else zr0
                        d2t = d21 if dh_ else d20
                        nc.vector.tensor_mul(
                            zrt[:, j0 : j0 + n], ps[:, done : done + n],
                            d2t[:, j0 : j0 + n],
                        )
                        done += n

                # drain both halves in MMCH chunks so mul/sigmoid/z-matmul/
                # zr-mul pipeline across engines
                for m0 in range(0, NSH - H0, MMCH):
                    w = min(MMCH, NSH - H0 - m0)
                    sl = slice(m0, m0 + w)
                    nc.vector.tensor_mul(s1[:, sl], acc1[:, sl], dinvb[:, H0 + m0 : H0 + m0 + w])
                    nc.scalar.activation(
                        s1[:, sl], s1[:, sl], mybir.ActivationFunctionType.Sigmoid,
                        bias=b1[:, 0:1],
                    )
                    ps = psz.tile([1, MMCH], f32, tag="psz")
                    nc.tensor.matmul(
                        ps[:, :w], w2[:], s1[:, sl], start=True, stop=True
                    )
                    _zr_write(ps, H0 + m0, w)
                for m0 in range(0, H0, MMCH):
                    sl = slice(m0, m0 + MMCH)
                    nc.vector.tensor_mul(s0[:, sl], ps_h0[:, sl], dinvb[:, sl])
                    nc.scalar.activation(
                        s0[:, sl], s0[:, sl], mybir.ActivationFunctionType.Sigmoid,
                        bias=b1[:, 0:1],
                    )
                    ps = psz.tile([1, MMCH], f32, tag="psz")
                    nc.tensor.matmul(ps[:], w2[:], s0[:, sl], start=True, stop=True)
                    _zr_write(ps, m0, MMCH)
                nc.sync.dma_start(out=zin[0:1, :], in_=zr0[:])
                nc.scalar.dma_start(out=zin[1:2, :], in_=zr1[:])

        nc.gpsimd.collective_compute(
            "AllGather", mybir.AluOpType.bypass,
            replica_groups=[list(range(NCORES))],
            ins=[zin[:].opt()], outs=[zall[:].opt()],
        )

        # ---- layer 2 ----
        with (
            tc.tile_pool(name="k2pool", bufs=1) as pool2,
            tc.tile_pool(name="ps2pool", bufs=1, space="PSUM") as ps2pool,
        ):
            zt = pool2.tile([P, K2W], f32)
            ztb = pool2.tile([P, K2W], bf16)
            g2 = pool2.tile([P, SQ2], f32)
            eidx2 = pool2.tile([P, SQ2 // 16], i16)
            perm2 = pool2.tile([P, DHP // 16], i16)
            accp2 = pool2.tile([P, P2], f32)
            t2 = pool2.tile([P, DHP], f32)
            o2 = pool2.tile([2, DHP], f32)
            zrTb = pool2.tile([2, DHP], bf16)
            zrT = pool2.tile([2, DHP], f32)
            dinvd2T = pool2.tile([2, DHP], f32)
            ps2 = ps2pool.tile([2, DHP], f32)
            # zero ztb in full (garbage partitions must be 0.0, not junk, so
            # the 0-weighted rows of the wones matmul cannot poison the sum);
            # runs on Act during the collective window
            nc.scalar.memzero(ztb[:])
            nc.vector.memset(accp2[:, 0:1], 0.0)
            # local z' + dst dinv in [2, DHP] layout for the final combine
            # (zin round-trip; overlaps the collective)
            nc.scalar.dma_start(out=zrTb[:], in_=zin[:])
            nc.scalar.dma_start(out=dinvd2T[:], in_=dinvd2_d[:])
            nc.scalar.dma_start(out=eidx2[:], in_=eidx2_d[:])
            nc.scalar.dma_start(out=perm2[:], in_=perm2_d[:])
            nc.scalar.activation(zrT[:], zrTb[:], mybir.ActivationFunctionType.Copy)
            # group 2*sq + dh holds z' of src quarter sq (dh in {0,1} share it),
            # loaded as bf16 [4, 4*DHP] from zall (zero col lives at 4*DHP);
            # split into col chunks over the 3 DMA-capable queues, then
            # converted to the f32 gather table (Act + DVE halves)
            ZC = ((4 * DHP + 2) // 3 + 15) // 16 * 16  # 4192: 3 col chunks
            zq = [nc.sync, nc.scalar, nc.gpsimd]
            for i in range(3):
                c0 = i * ZC
                cw = min(ZC, 4 * DHP - c0)
                zq[i].dma_start(
                    out=ztb[0:P:32, c0 : c0 + cw], in_=zall[:, c0 : c0 + cw]
                )
                zq[(i + 1) % 3].dma_start(
                    out=ztb[16:P:32, c0 : c0 + cw], in_=zall[:, c0 : c0 + cw]
                )
            for i in range(3):
                c0 = i * ZC
                cw = (min(ZC, K2W - c0) if i < 2 else K2W - c0)
                if i < 2:
                    nc.scalar.activation(
                        zt[:, c0 : c0 + cw], ztb[:, c0 : c0 + cw],
                        mybir.ActivationFunctionType.Copy,
                    )
                else:
                    nc.vector.tensor_copy(zt[:, c0:], ztb[:, c0:])
            nc.gpsimd.ap_gather(
                g2[:], zt[:], eidx2[:], channels=P, num_elems=K2W, d=1, num_idxs=SQ2
            )
            # ladder reduce, split across engines: k=1 -> Act copy, small k ->
            # gpsimd strided adds (Pool is idle here), rest -> DVE reduce
            for (_, off, n_rows, k, col) in descr2:
                dst_ap = accp2[:, col : col + n_rows]
                if k == 1:
                    nc.scalar.activation(
                        dst_ap, g2[:, off : off + n_rows],
                        mybir.ActivationFunctionType.Copy,
                    )
                elif k in (2, 3):
                    src3 = g2[:, off : off + n_rows * k].rearrange(
                        "p (a b) -> p a b", a=n_rows, b=k
                    )
                    nc.gpsimd.tensor_add(dst_ap, src3[:, :, 0], src3[:, :, 1])
                    for j in range(2, k):
                        nc.gpsimd.tensor_add(dst_ap, dst_ap, src3[:, :, j])
                else:
                    nc.vector.tensor_reduce(
                        dst_ap,
                        g2[:, off : off + n_rows * k].rearrange(
                            "p (a b) -> p a b", a=n_rows, b=k
                        ),
                        axis=mybir.AxisListType.X, op=mybir.AluOpType.add,
                    )
            # keep the PE p-state warm through the gather/reduce window so the
            # real matmuls below run at full clock: dummies chain on the zt
            # load, then on reduce outputs (accp2 slices) to span the window
            psd = ps2pool.tile([1, MMCH], f32)
            for i in range(4):
                nc.tensor.matmul(
                    psd[:], w2[:], zt[:, i * MMCH : (i + 1) * MMCH],
                    start=True, stop=True,
                )
            for i in range(9):
                nc.tensor.matmul(
                    psd[:], w2[:], accp2[:, (i % 6) * MMCH : (i % 6 + 1) * MMCH],
                    start=True, stop=True,
                )
            nc.gpsimd.ap_gather(
                t2[:], accp2[:], perm2[:], channels=P, num_elems=P2, d=1, num_idxs=DHP
            )
            # out2 = sigmoid(dinv_dst * (gathered sum + local self-loop z') + b2)
            # chunked so add/mul/sigmoid/DMA pipeline behind the matmuls
            for m0 in range(0, DHP, MMCH):
                w = min(MMCH, DHP - m0)
                sl = slice(m0, m0 + w)
                nc.tensor.matmul(
                    ps2[:, sl], wones[:], t2[:, sl], start=True, stop=True
                )
                nc.vector.tensor_add(o2[:, sl], ps2[:, sl], zrT[:, sl])
                nc.vector.tensor_mul(o2[:, sl], o2[:, sl], dinvd2T[:, sl])
                nc.scalar.activation(
                    o2[:, sl], o2[:, sl], mybir.ActivationFunctionType.Sigmoid, bias=b2
                )
            nc.sync.dma_start(out=out_d[:, :DHP // 2], in_=o2[:, :DHP // 2])
            nc.scalar.dma_start(out=out_d[:, DHP // 2 :], in_=o2[:, DHP // 2 :])
    nc.finalize()
    return nc


def _sim_ns(nc):
    from concourse import bass_interp

    sim = bass_interp.CoreSim(nc, no_exec=True, publish_trace=False)
    sim.simulate()
    return int(sim.time)


def _assemble_out(results):
    out = np.zeros((N, 1), dtype=np.float32)
    for c in range(NCORES):
        o = results[c]["out2"]  # [2, DHP]
        out[c * NSH : c * NSH + DH, 0] = o[0, :DH]
        out[c * NSH + DH : (c + 1) * NSH, 0] = o[1, :DH]
    return out


def kernel(x, edge_index, W1, b1, W2, b2):
    global LAST_SIM_NS
    x = np.asarray(x, dtype=np.float32)
    edge_index = np.asarray(edge_index)
    inputs, meta = host_prep(x, edge_index, np.asarray(W1), b1, W2, b2)
    nc = build_fused(meta)
    if MEASURE:
        LAST_SIM_NS = _sim_ns(nc)
    res = run_bass_kernel_spmd(nc, inputs, list(range(NCORES)))
    return _assemble_out(res.results)


# revision 3
# speedup vs baseline: 1.0272x; 1.0040x over previous
"""2-layer GCN (PyG GCNConv x2 + sigmoid) on 8 TRN2 NeuronCores, single fused NEFF.

Cost-model-driven design (777965ns baseline -> 255916ns):
- ap_gather costs max(table_width, num_idxs)*0.833ns on GPSIMD; all gathers
  are sized so num_idxs >= table width (~0.833ns/edge-slot).
- Layer 1, dst-sharded: 8 src-octant tables [128, 6256] built by bf16
  matmuls (1 PE cycle/row; x uploaded pre-scaled by dinv[src] as bf16);
  per octant one ~14k-idx gather pass (2 chunks) + degree-ladder
  tensor_reduce (k=1 buckets are plain copies -> Activation engine).
- Assembly is software-pipelined one octant behind the gathers (accp double-
  buffered) so the in-order Pool queue never stalls on DVE reduces. Per
  octant: perm-gather to dst order, then nodes [0,3072) accumulate over
  octants in PSUM via identity matmul on the PE, the rest via DVE adds
  (PSUM cannot hold 6250 f32 cols next to the matmul staging bank).
- Finalize pipelines mul/sigmoid/z-matmul/z'-scale in 512-col chunks across
  DVE/Act/PE; z' rows are bf16 and AllGathered as one 12.5KB row pair per
  core (15us collective constant + payload at 40GB/s).
- Layer 2 is src-grouped: the 8 gpsimd 16-partition groups hold (src
  quarter, dst half) z' tables loaded bf16 in parallel on the 3 DMA queues
  and converted to f32 on Act+DVE; one gather covers all ~14k layer-2 slots;
  ladder reduce is split k=1->Act, k<=5->gpsimd strided adds, rest->DVE;
  cross-quarter combine is a 0/1-stationary PE matmul into [2, 3136];
  appended self-loops skip the gather entirely (their z'[n] term is the
  local z row, added in the finale). Dummy matmuls keep the PE p-state warm
  across gather windows. DMA cost is per-partition-bytes * 0.3855ns, so wide
  loads are split across the sync/scalar/gpsimd queues.
"""

import sys

sys.path.insert(0, "/opt/trn_rl_repo")
import numpy as np
from contextlib import ExitStack

from concourse import bacc, mybir
from concourse.tile import TileContext
from concourse.bass_utils import run_bass_kernel_spmd

try:
    import ml_dtypes

    _BF16 = np.dtype(ml_dtypes.bfloat16)
except Exception:  # pragma: no cover
    _BF16 = None

MEASURE = False
LAST_SIM_NS = None

N = 50000
E = 800000
F = 128
P = 128
NCORES = 8
NSH = N // NCORES  # 6250 dst nodes per core
NO = 8  # src octants (tables)
ON = N // NO  # 6250 src nodes per octant
OCOLS = 6256  # octant table cols: [zero, 6250 nodes, pad] mult of 16
PERM_NI = 6256  # pad16(NSH): assembly perm idx count per octant
H0 = 3072  # nodes 0..H0-1 accumulate in PSUM via PE
H1 = PERM_NI - H0  # 3200 (covers nodes H0..6249 + pad)
GMAX = 7808  # max L1 gather chunk (slots)
MMCH = 512  # matmul moving chunk
K2G = 8  # layer-2 partition groups = (src quarter, dst half)
DH = NSH // 2  # 3125 dst nodes per k2 half
DHP = 3136  # pad16(DH)
K2W = 4 * DHP + 16  # k2 table cols: [zero, 4 half-rows of 3136, pad]


def _wrap16(idx_flat):
    n = idx_flat.shape[0]
    assert n % 16 == 0
    return np.ascontiguousarray(idx_flat.reshape(n // 16, 16).T)


def _pad16(n, mult=16):
    return ((n + mult - 1) // mult) * mult


def _concat_aranges(lens):
    if len(lens) == 0:
        return np.zeros(0, dtype=np.int64)
    total = int(lens.sum())
    out = np.ones(total, dtype=np.int64)
    ends = np.cumsum(lens)
    out[0] = 0
    out[ends[:-1]] = -(lens[:-1] - 1)
    return np.cumsum(out)


def _bucket_lut(kmax, exact, buckets):
    lut = np.arange(max(kmax + 1, exact + 1))
    for kk in range(exact + 1, len(lut)):
        for bb in buckets:
            if kk <= bb:
                lut[kk] = bb
                break
        else:
            lut[kk] = ((kk + 63) // 64) * 64
    return lut


def _ladder_layout(kapb_all, cap_fn):
    """kapb_all: [NCORES, NCELLS, NNODES] bucketed kappas sharing one layout.
    Returns (budgets {k: n}, chunks list, descr [(ch, off, n_rows, k, col)],
    n_cols)."""
    b = {}
    for k in np.unique(kapb_all):
        k = int(k)
        if k == 0:
            continue
        nk = int((kapb_all == k).sum(axis=-1).max())
        if nk > 0:
            b[k] = nk
    raw = sum(k * n for k, n in b.items())
    cap = cap_fn(raw)
    descr = []
    col = 1
    ch, off = 0, 0
    for k in sorted(b):
        left = b[k]
        while left > 0:
            fit = min(left, (cap - off) // k)
            if fit == 0:
                ch += 1
                off = 0
                fit = min(left, cap // k)
            descr.append((ch, off, fit, k, col))
            off += fit * k
            col += fit
            left -= fit
    n_chunks = ch + 1
    return b, n_chunks, cap, descr, col


def _fill_slots(kv, lut, descr, col2k_base, s_sorted, cap):
    """Place each node's edges into its ladder row. kv: per-node actual count;
    s_sorted: edge values sorted by node. Returns (slot_positions, values,
    node_cols)."""
    kvb = lut[kv]
    nodes = np.nonzero(kv)[0]
    kn = kv[nodes]
    knb = kvb[nodes]
    nd = np.lexsort((nodes, knb))
    nodes_s, kn_s, knb_s = nodes[nd], kn[nd], knb[nd]
    rank = np.zeros(len(nodes_s), dtype=np.int64)
    colof = np.zeros(len(nodes_s), dtype=np.int64)
    for k in np.unique(knb_s):
        mk = knb_s == k
        rank[mk] = np.arange(mk.sum())
        colof[mk] = col2k_base[int(k)]
    node_col = colof + rank
    ncols = max(d[4] + d[2] for d in descr)
    col2slot = np.full(ncols, -1, dtype=np.int64)
    for ch, off, n_rows, k, col in descr:
        cols = np.arange(n_rows)
        col2slot[col + cols] = ch * cap + off + cols * k
    starts = col2slot[node_col]
    eslots = np.repeat(starts, kn_s) + _concat_aranges(kn_s)
    ptr = np.zeros(len(kv) + 1, dtype=np.int64)
    ptr[1:] = np.cumsum(kv)
    ev = (
        np.concatenate([s_sorted[ptr[n] : ptr[n + 1]] for n in nodes_s])
        if len(nodes_s)
        else np.zeros(0, dtype=np.int64)
    )
    pm = np.zeros(len(kv), dtype=np.int16)
    pm[nodes_s] = node_col.astype(np.int16)
    return eslots, ev, pm


def host_prep(x, edge_index, W1, b1, W2, b2):
    src = np.concatenate([edge_index[0], np.arange(N, dtype=np.int64)]).astype(np.int32)
    dst = np.concatenate([edge_index[1], np.arange(N, dtype=np.int64)]).astype(np.int32)
    deg = np.bincount(dst, minlength=N).astype(np.float32)
    dinv = 1.0 / np.sqrt(np.maximum(deg, 1e-12))
    dinv[deg <= 0] = 0.0

    # random node->table-position permutation balances per-(core,octant)
    # degree distributions, keeping shared max-over-core budgets tight
    psrc = np.random.default_rng(12345).permutation(N)
    pinv = np.argsort(psrc)

    xtp = (x * dinv[:, None]).T.astype(np.float32)[:, pinv]  # [128, N] pos order
    xt = np.zeros((P, NO * OCOLS), dtype=np.float32)
    for o in range(NO):
        xt[:, o * OCOLS + 1 : o * OCOLS + 1 + ON] = xtp[:, o * ON : (o + 1) * ON]
    xt_bf16 = xt.astype(_BF16)

    core = dst // NSH
    dstl = dst % NSH
    pos = psrc[src]
    octant = pos // ON
    srcl = (pos % ON).astype(np.int64) + 1

    # kappa per (core, octant, local dst node)
    kap = np.zeros((NCORES, NO, NSH), dtype=np.int32)
    for c in range(NCORES):
        mc = core == c
        for o in range(NO):
            m = mc & (octant == o)
            kap[c, o] = np.bincount(dstl[m], minlength=NSH)

    kmax = int(kap.max())
    lut = _bucket_lut(kmax, 12, (14, 16, 19, 22, 26, 32, 40, 48, 64, 96, 128, 192, 256))
    kapb = lut[kap]

    layouts = []  # per octant: (n_chunks, cap, descr, n_cols, kbase)
    for o in range(NO):
        b, n_chunks, cap, descr, ncol = _ladder_layout(
            kapb[:, o, :], lambda raw: min(GMAX, _pad16((raw + 1) // 2 + 64))
        )
        kbase = {}
        for ch, off, n_rows, k, col in descr:
            kbase.setdefault(k, col)
        layouts.append((n_chunks, cap, descr, ncol, kbase))

    SQ = [layouts[o][0] * layouts[o][1] for o in range(NO)]  # slots per octant
    PQ = _pad16(max(layouts[o][3] for o in range(NO)))

    order = np.lexsort((dstl, octant, core))
    so, do_, oo, co = srcl[order], dstl[order], octant[order], core[order]
    eidx = np.zeros((NCORES, sum(SQ)), dtype=np.int16)
    perms = np.zeros((NCORES, NO, PERM_NI), dtype=np.int16)
    for c in range(NCORES):
        obase = 0
        for o in range(NO):
            m = (co == c) & (oo == o)
            _, cap, descr, _, kbase = layouts[o]
            eslots, ev, pm = _fill_slots(kap[c, o], lut, descr, kbase, so[m], cap)
            eidx[c, obase + eslots] = ev.astype(np.int16)
            perms[c, o, :NSH] = pm
            obase += SQ[o]

    eidx_w = np.zeros((NCORES, P, sum(SQ) // 16), dtype=np.int16)
    perm_w = np.zeros((NCORES, P, NO * (PERM_NI // 16)), dtype=np.int16)
    for c in range(NCORES):
        eidx_w[c] = np.tile(_wrap16(eidx[c]), (K2G, 1))
        pw = np.concatenate([_wrap16(perms[c, o]) for o in range(NO)], axis=1)
        perm_w[c] = np.tile(pw, (K2G, 1))

    # ---------------- layer 2 (src-grouped) ----------------
    # Appended self-loops are excluded (their z'[n] term is added on-device
    # from the local z row); only the original E edges go through the gather.
    # group g = 2*src_quarter + dst_half; table per group: z' of src quarter
    # laid out as 4 half-rows of DHP (cores 2sq,2sq+1 x dst-halves), matching
    # the AllGather result zall [16, DHP].
    src2 = src[:E]
    dst2 = dst[:E]
    core2 = dst2 // NSH
    dstl2 = dst2 % NSH
    srcq = (src2 // (2 * NSH)).astype(np.int64)  # 0..3
    _c2 = (src2 // NSH).astype(np.int64) % 2  # core parity within quarter
    _i2 = (src2 % NSH).astype(np.int64)
    srcl2 = (2 * _c2 + _i2 // DH) * DHP + (_i2 % DH)  # table position (0-based)
    dh = dstl2 // DH  # 0..1
    j2 = dstl2 % DH  # 0..3124
    grp = 2 * srcq + dh

    kap2 = np.zeros((NCORES, K2G, DH), dtype=np.int32)
    for c in range(NCORES):
        mc = core2 == c
        for g in range(K2G):
            m = mc & (grp == g)
            kap2[c, g] = np.bincount(j2[m], minlength=DH)

    kmax2 = int(kap2.max())
    lut2 = _bucket_lut(kmax2, 9, (11, 13, 15, 18, 22, 27, 33, 40, 48, 64, 96, 128, 192, 256))
    kapb2 = lut2[kap2]
    b2_, n_chunks2, cap2, descr2, ncol2 = _ladder_layout(
        kapb2.reshape(NCORES, K2G, DH), lambda raw: _pad16(raw)
    )
    assert n_chunks2 == 1
    SQ2 = cap2
    P2 = _pad16(ncol2)
    kbase2 = {}
    for ch, off, n_rows, k, col in descr2:
        kbase2.setdefault(k, col)

    order2 = np.lexsort((j2, grp, core2))
    so2, jo2, go2, co2 = srcl2[order2], j2[order2], grp[order2], core2[order2]
    eidx2 = np.full((NCORES, K2G, SQ2), 4 * DHP, dtype=np.int16)  # pad -> zero col
    perm2 = np.zeros((NCORES, K2G, DHP), dtype=np.int16)
    for c in range(NCORES):
        for g in range(K2G):
            m = (co2 == c) & (go2 == g)
            eslots, ev, pm = _fill_slots(kap2[c, g], lut2, descr2, kbase2, so2[m], cap2)
            eidx2[c, g, eslots] = ev.astype(np.int16)
            perm2[c, g, :DH] = pm

    eidx2_w = np.zeros((NCORES, P, SQ2 // 16), dtype=np.int16)
    perm2_w = np.zeros((NCORES, P, DHP // 16), dtype=np.int16)
    for c in range(NCORES):
        for g in range(K2G):
            eidx2_w[c, g * 16 : (g + 1) * 16] = _wrap16(eidx2[c, g])
            perm2_w[c, g * 16 : (g + 1) * 16] = _wrap16(perm2[c, g])

    ident = np.eye(P, dtype=np.float32)
    wones = np.zeros((P, 2), dtype=np.float32)
    for g in range(K2G):
        wones[16 * g, g % 2] = 1.0

    dinvb = np.zeros((NCORES, P, NSH), dtype=np.float32)
    dinvd2 = np.zeros((NCORES, 2, DHP), dtype=np.float32)
    for c in range(NCORES):
        dsh = dinv[c * NSH : (c + 1) * NSH]
        dinvb[c] = np.tile(dsh, (P, 1))
        dinvd2[c, 0, :DH] = dsh[:DH]
        dinvd2[c, 1, :DH] = dsh[DH:]

    meta = dict(layouts=layouts, SQ=SQ, PQ=PQ, descr2=descr2, SQ2=SQ2, P2=P2,
                b2=float(np.asarray(b2).reshape(-1)[0]))
    inputs = []
    for c in range(NCORES):
        inputs.append(
            {
                "xt": xt_bf16,
                "w1": W1.astype(np.float32).astype(_BF16),
                "b1": np.asarray(b1, dtype=np.float32).reshape(P, 1),
                "w2": np.asarray(W2, dtype=np.float32).reshape(P, 1),
                "ident": ident,
                "wones": wones,
                "eidx": np.ascontiguousarray(eidx_w[c]),
                "perm": np.ascontiguousarray(perm_w[c]),
                "eidx2": np.ascontiguousarray(eidx2_w[c]),
                "perm2": np.ascontiguousarray(perm2_w[c]),
                "dinvb": np.ascontiguousarray(dinvb[c]),
                "dinvd2": np.ascontiguousarray(dinvd2[c]),
            }
        )
    return inputs, meta


def build_fused(meta):
    layouts, SQ, PQ = meta["layouts"], meta["SQ"], meta["PQ"]
    descr2, SQ2, P2 = meta["descr2"], meta["SQ2"], meta["P2"]
    b2 = meta["b2"]
    nc = bacc.Bacc(None, target_bir_lowering=False)
    f32, bf16, i16 = mybir.dt.float32, mybir.dt.bfloat16, mybir.dt.int16

    xt_d = nc.dram_tensor("xt", [P, NO * OCOLS], bf16, kind="ExternalInput")
    w1_d = nc.dram_tensor("w1", [P, P], bf16, kind="ExternalInput")
    b1_d = nc.dram_tensor("b1", [P, 1], f32, kind="ExternalInput")
    w2_d = nc.dram_tensor("w2", [P, 1], f32, kind="ExternalInput")
    ident_d = nc.dram_tensor("ident", [P, P], f32, kind="ExternalInput")
    wones_d = nc.dram_tensor("wones", [P, 2], f32, kind="ExternalInput")
    eidx_d = nc.dram_tensor("eidx", [P, sum(SQ) // 16], i16, kind="ExternalInput")
    perm_d = nc.dram_tensor("perm", [P, NO * (PERM_NI // 16)], i16, kind="ExternalInput")
    eidx2_d = nc.dram_tensor("eidx2", [P, SQ2 // 16], i16, kind="ExternalInput")
    perm2_d = nc.dram_tensor("perm2", [P, DHP // 16], i16, kind="ExternalInput")
    dinvb_d = nc.dram_tensor("dinvb", [P, NSH], f32, kind="ExternalInput")
    dinvd2_d = nc.dram_tensor("dinvd2", [2, DHP], f32, kind="ExternalInput")
    out_d = nc.dram_tensor("out2", [2, DHP], f32, kind="ExternalOutput")
    zin = nc.dram_tensor("zin_cc", [2, DHP], bf16, kind="Internal")
    zall = nc.dram_tensor("zall_cc", [4, 4 * DHP], bf16, kind="Internal", addr_space="Shared")

    with ExitStack() as ctx:
        tc = ctx.enter_context(TileContext(nc))
        cpool = ctx.enter_context(tc.tile_pool(name="cpool", bufs=1))
        w1 = cpool.tile([P, P], bf16)
        b1 = cpool.tile([P, 1], f32)
        w2 = cpool.tile([P, 1], f32)
        ident = cpool.tile([P, P], f32)
        wones = cpool.tile([P, 2], f32)
        perm = cpool.tile([P, NO * (PERM_NI // 16)], i16)
        nc.scalar.dma_start(out=w1[:], in_=w1_d[:])
        nc.scalar.dma_start(out=b1[:], in_=b1_d[:])
        nc.scalar.dma_start(out=w2[:], in_=w2_d[:])
        nc.scalar.dma_start(out=ident[:], in_=ident_d[:])
        nc.scalar.dma_start(out=wones[:], in_=wones_d[:])
        nc.scalar.dma_start(out=perm[:], in_=perm_d[:])


        with (
            tc.tile_pool(name="apool", bufs=1) as apool,
            tc.tile_pool(name="pshpool", bufs=1, space="PSUM") as pshpool,
        ):
            acc1 = apool.tile([P, H1], f32)
            ps_h0 = pshpool.tile([P, H0], f32)

            with (
                tc.tile_pool(name="tabs", bufs=2) as tabs,
                tc.tile_pool(name="xpool", bufs=2) as xpool,
                tc.tile_pool(name="epool", bufs=2) as epool,
                tc.tile_pool(name="gpool", bufs=2) as gpool,
                tc.tile_pool(name="appool", bufs=2) as appool,
                tc.tile_pool(name="tpool", bufs=1) as tpool,
                tc.tile_pool(name="pstab", bufs=1, space="PSUM") as pstab,
            ):
                sq_base = 0
                SQMAX = max(SQ)
                CAPMAX = max(layouts[o][1] for o in range(NO))
                prev_accp = None  # assembly for octant o runs after gathers o+1

                def _assemble(o, accp_o):
                    # perm-gather accp -> dst order, then accumulate: nodes
                    # [0,H0) into PSUM via identity matmul, [H0,) via DVE adds
                    if o == NO - 1:
                        # keep PE warm across the reduce+perm wait so the
                        # final assembly matmuls run at full clock
                        pst = pstab.tile([P, 2 * MMCH], f32, tag="pstab")
                        for i in range(5):
                            nc.tensor.matmul(
                                pst[0:1, :MMCH], w2[:],
                                accp_o[:, (i % 8) * MMCH : (i % 8 + 1) * MMCH],
                                start=True, stop=True,
                            )
                    t = tpool.tile([P, PERM_NI], f32, tag="t")
                    pbase = o * (PERM_NI // 16)
                    nc.gpsimd.ap_gather(
                        t[:], accp_o[:], perm[:, pbase : pbase + PERM_NI // 16],
                        channels=P, num_elems=PQ, d=1, num_idxs=PERM_NI,
                    )
                    for m0 in range(0, H0, MMCH):
                        nc.tensor.matmul(
                            ps_h0[:, m0 : m0 + MMCH], ident[:], t[:, m0 : m0 + MMCH],
                            start=(o == 0), stop=(o == NO - 1), skip_group_check=True,
                        )
                    if o == 0:
                        nc.scalar.activation(
                            acc1[:], t[:, H0:], mybir.ActivationFunctionType.Copy
                        )
                    else:
                        nc.vector.tensor_add(acc1[:], acc1[:], t[:, H0:])

                for o in range(NO):
                    n_chunks, cap, descr, _, _ = layouts[o]
                    tab = tabs.tile([P, OCOLS], f32, tag="tab")
                    XB = 1568 if o == 0 else 784
                    for x0 in range(0, OCOLS, XB):
                        xw = min(XB, OCOLS - x0)
                        xc = xpool.tile([P, 1568 if o == 0 else 784], bf16, tag="x")
                        nc.sync.dma_start(
                            out=xc[:, :xw], in_=xt_d[:, o * OCOLS + x0 : o * OCOLS + x0 + xw]
                        )
                        for m0 in range(0, xw, 2 * MMCH):
                            mw = min(2 * MMCH, xw - m0)
                            ps = pstab.tile([P, 2 * MMCH], f32, tag="pstab")
                            for h in range(0, mw, MMCH):
                                hw_ = min(MMCH, mw - h)
                                nc.tensor.matmul(
                                    ps[:, h : h + hw_], w1[:], xc[:, m0 + h : m0 + h + hw_],
                                    start=True, stop=True,
                                )
                            nc.scalar.activation(
                                tab[:, x0 + m0 : x0 + m0 + mw], ps[:, :mw],
                                mybir.ActivationFunctionType.Copy,
                            )
                    # gather + ladder reduce (k=1 buckets are copies -> Act)
                    eb = epool.tile([P, SQMAX // 16], i16, tag="eidx")
                    (nc.gpsimd if o == 0 else nc.sync).dma_start(
                        out=eb[:, : SQ[o] // 16],
                        in_=eidx_d[:, sq_base // 16 : (sq_base + SQ[o]) // 16],
                    )
                    accp = appool.tile([P, PQ], f32, tag="accp")
                    nc.vector.memset(accp[:, 0:1], 0.0)
                    by_chunk = {}
                    for d_ in descr:
                        by_chunk.setdefault(d_[0], []).append(d_)
                    for ch in range(n_chunks):
                        g = gpool.tile([P, CAPMAX], f32, tag="g")
                        nc.gpsimd.ap_gather(
                            g[:, :cap], tab[:], eb[:, ch * cap // 16 : (ch + 1) * cap // 16],
                            channels=P, num_elems=OCOLS, d=1, num_idxs=cap,
                        )
                        for (_, off, n_rows, k, col) in by_chunk.get(ch, []):
                            if k == 1:
                                nc.scalar.activation(
                                    accp[:, col : col + n_rows], g[:, off : off + n_rows],
                                    mybir.ActivationFunctionType.Copy,
                                )
                            else:
                                nc.vector.tensor_reduce(
                                    accp[:, col : col + n_rows],
                                    g[:, off : off + n_rows * k].rearrange(
                                        "p (a b) -> p a b", a=n_rows, b=k
                                    ),
                                    axis=mybir.AxisListType.X, op=mybir.AluOpType.add,
                                )
                    if prev_accp is not None:
                        _assemble(o - 1, prev_accp)
                    prev_accp = accp
                    sq_base += SQ[o]
                _assemble(NO - 1, prev_accp)

            # finalize layer 1: sigma1 = sigmoid(dinv*acc + b1); z' = dinv * W2^T sigma1
            # z' is kept as two 1-partition rows (dst halves at DH=3125, padded
            # to DHP), scaled by dinvd2 on write, then DMA'd to zin [2, DHP]
            with (
                tc.tile_pool(name="fin", bufs=1) as fin,
                tc.tile_pool(name="psz", bufs=2, space="PSUM") as psz,
            ):
                s0 = fin.tile([P, H0], f32)
                s1 = fin.tile([P, NSH - H0], f32)
                dinvb = fin.tile([P, NSH], f32)
                zr0 = fin.tile([1, DHP], bf16)
                zr1 = fin.tile([1, DHP], bf16)
                d20 = fin.tile([1, DHP], f32)
                d21 = fin.tile([1, DHP], f32)
                nc.scalar.dma_start(out=dinvb[:, H0:], in_=dinvb_d[:, H0:])
                nc.sync.dma_start(out=dinvb[:, :H0], in_=dinvb_d[:, :H0])
                nc.sync.dma_start(out=d20[:], in_=dinvd2_d[0:1, :])
                nc.sync.dma_start(out=d21[:], in_=dinvd2_d[1:2, :])
                nc.vector.memset(zr0[:, DH:], 0.0)
                nc.vector.memset(zr1[:, DH:], 0.0)

                def _zr_write(ps, glob0, w):
                    # ps[0, :w] holds z~ for global nodes [glob0, glob0+w)
                    done = 0
                    while done < w:
                        g0 = glob0 + done
                        dh_ = g0 // DH
                        j0 = g0 - dh_ * DH
                        n = min(w - done, DH - j0)
                        zrt = zr1 if dh_ else zr0
                        d2t = d21 if dh_ else d20
                        nc.vector.tensor_mul(
                            zrt[:, j0 : j0 + n], ps[:, done : done + n],
                            d2t[:, j0 : j0 + n],
                        )
                        done += n

                # drain both halves in MMCH chunks so mul/sigmoid/z-matmul/
                # zr-mul pipeline across engines
                for m0 in range(0, NSH - H0, MMCH):
                    w = min(MMCH, NSH - H0 - m0)
                    sl = slice(m0, m0 + w)
                    nc.vector.tensor_mul(s1[:, sl], acc1[:, sl], dinvb[:, H0 + m0 : H0 + m0 + w])
                    nc.scalar.activation(
                        s1[:, sl], s1[:, sl], mybir.ActivationFunctionType.Sigmoid,
                        bias=b1[:, 0:1],
                    )
                    ps = psz.tile([1, MMCH], f32, tag="psz")
                    nc.tensor.matmul(
                        ps[:, :w], w2[:], s1[:, sl], start=True, stop=True
                    )
                    _zr_write(ps, H0 + m0, w)
                for m0 in range(0, H0, MMCH):
                    sl = slice(m0, m0 + MMCH)
                    nc.vector.tensor_mul(s0[:, sl], ps_h0[:, sl], dinvb[:, sl])
                    nc.scalar.activation(
                        s0[:, sl], s0[:, sl], mybir.ActivationFunctionType.Sigmoid,
                        bias=b1[:, 0:1],
                    )
                    ps = psz.tile([1, MMCH], f32, tag="psz")
                    nc.tensor.matmul(ps[:], w2[:], s0[:, sl], start=True, stop=True)
                    _zr_write(ps, m0, MMCH)
                nc.sync.dma_start(out=zin[0:1, :], in_=zr0[:])
                nc.scalar.dma_start(out=zin[1:2, :], in_=zr1[:])

        nc.gpsimd.collective_compute(
            "AllGather", mybir.AluOpType.bypass,
            replica_groups=[list(range(NCORES))],
            ins=[zin[:].opt()], outs=[zall[:].opt()],
        )

        # ---- layer 2 ----
        with (
            tc.tile_pool(name="k2pool", bufs=1) as pool2,
            tc.tile_pool(name="ps2pool", bufs=1, space="PSUM") as ps2pool,
        ):
            zt = pool2.tile([P, K2W], f32)
            ztb = pool2.tile([P, K2W], bf16)
            g2 = pool2.tile([P, SQ2], f32)
            eidx2 = pool2.tile([P, SQ2 // 16], i16)
            perm2 = pool2.tile([P, DHP // 16], i16)
            accp2 = pool2.tile([P, P2], f32)
            t2 = pool2.tile([P, DHP], f32)
            o2 = pool2.tile([2, DHP], f32)
            zrTb = pool2.tile([2, DHP], bf16)
            zrT = pool2.tile([2, DHP], f32)
            dinvd2T = pool2.tile([2, DHP], f32)
            ps2 = ps2pool.tile([2, DHP], f32)
            # zero ztb in full (garbage partitions must be 0.0, not junk, so
            # the 0-weighted rows of the wones matmul cannot poison the sum);
            # runs on Act during the collective window
            nc.scalar.memzero(ztb[:])
            nc.vector.memset(accp2[:, 0:1], 0.0)
            # local z' + dst dinv in [2, DHP] layout for the final combine
            # (zin round-trip; overlaps the collective)
            nc.scalar.dma_start(out=zrTb[:], in_=zin[:])
            nc.scalar.dma_start(out=dinvd2T[:], in_=dinvd2_d[:])
            nc.scalar.dma_start(out=eidx2[:], in_=eidx2_d[:])
            nc.scalar.dma_start(out=perm2[:], in_=perm2_d[:])
            nc.scalar.activation(zrT[:], zrTb[:], mybir.ActivationFunctionType.Copy)
            # group 2*sq + dh holds z' of src quarter sq (dh in {0,1} share it),
            # loaded as bf16 [4, 4*DHP] from zall (zero col lives at 4*DHP);
            # split into col chunks over the 3 DMA-capable queues, then
            # converted to the f32 gather table (Act + DVE halves)
            ZC = ((4 * DHP + 2) // 3 + 15) // 16 * 16  # 4192: 3 col chunks
            zq = [nc.sync, nc.scalar, nc.gpsimd]
            for i in range(3):
                c0 = i * ZC
                cw = min(ZC, 4 * DHP - c0)
                zq[i].dma_start(
                    out=ztb[0:P:32, c0 : c0 + cw], in_=zall[:, c0 : c0 + cw]
                )
                zq[(i + 1) % 3].dma_start(
                    out=ztb[16:P:32, c0 : c0 + cw], in_=zall[:, c0 : c0 + cw]
                )
            for i in range(3):
                c0 = i * ZC
                cw = (min(ZC, K2W - c0) if i < 2 else K2W - c0)
                if i < 2:
                    nc.scalar.activation(
                        zt[:, c0 : c0 + cw], ztb[:, c0 : c0 + cw],
                        mybir.ActivationFunctionType.Copy,
                    )
                else:
                    nc.vector.tensor_copy(zt[:, c0:], ztb[:, c0:])
            nc.gpsimd.ap_gather(
                g2[:], zt[:], eidx2[:], channels=P, num_elems=K2W, d=1, num_idxs=SQ2
            )
            # ladder reduce, split across engines: k=1 -> Act copy, small k ->
            # gpsimd strided adds (Pool is idle here), rest -> DVE reduce
            for (_, off, n_rows, k, col) in descr2:
                dst_ap = accp2[:, col : col + n_rows]
                if k == 1:
                    nc.scalar.activation(
                        dst_ap, g2[:, off : off + n_rows],
                        mybir.ActivationFunctionType.Copy,
                    )
                elif k in (2, 3, 4, 5, 6):
                    src3 = g2[:, off : off + n_rows * k].rearrange(
                        "p (a b) -> p a b", a=n_rows, b=k
                    )
                    nc.gpsimd.tensor_add(dst_ap, src3[:, :, 0], src3[:, :, 1])
                    for j in range(2, k):
                        nc.gpsimd.tensor_add(dst_ap, dst_ap, src3[:, :, j])
                else:
                    nc.vector.tensor_reduce(
                        dst_ap,
                        g2[:, off : off + n_rows * k].rearrange(
                            "p (a b) -> p a b", a=n_rows, b=k
                        ),
                        axis=mybir.AxisListType.X, op=mybir.AluOpType.add,
                    )
            # keep the PE p-state warm through the gather/reduce window so the
            # real matmuls below run at full clock: dummies chain on the zt
            # load, then on reduce outputs (accp2 slices) to span the window
            psd = ps2pool.tile([1, MMCH], f32)
            for i in range(4):
                nc.tensor.matmul(
                    psd[:], w2[:], zt[:, i * MMCH : (i + 1) * MMCH],
                    start=True, stop=True,
                )
            for i in range(9):
                nc.tensor.matmul(
                    psd[:], w2[:], accp2[:, (i % 6) * MMCH : (i % 6 + 1) * MMCH],
                    start=True, stop=True,
                )
            nc.gpsimd.ap_gather(
                t2[:], accp2[:], perm2[:], channels=P, num_elems=P2, d=1, num_idxs=DHP
            )
            # out2 = sigmoid(dinv_dst * (gathered sum + local self-loop z') + b2)
            # chunked so add/mul/sigmoid/DMA pipeline behind the matmuls
            for m0 in range(0, DHP, MMCH):
                w = min(MMCH, DHP - m0)
                sl = slice(m0, m0 + w)
                nc.tensor.matmul(
                    ps2[:, sl], wones[:], t2[:, sl], start=True, stop=True
                )
                nc.vector.tensor_add(o2[:, sl], ps2[:, sl], zrT[:, sl])
                nc.vector.tensor_mul(o2[:, sl], o2[:, sl], dinvd2T[:, sl])
                nc.scalar.activation(
                    o2[:, sl], o2[:, sl], mybir.ActivationFunctionType.Sigmoid, bias=b2
                )
                (nc.sync if (m0 // MMCH) % 2 == 0 else nc.scalar).dma_start(
                    out=out_d[:, sl], in_=o2[:, sl]
                )
    nc.finalize()
    return nc


def _sim_ns(nc):
    from concourse import bass_interp

    sim = bass_interp.CoreSim(nc, no_exec=True, publish_trace=False)
    sim.simulate()
    return int(sim.time)


def _assemble_out(results):
    out = np.zeros((N, 1), dtype=np.float32)
    for c in range(NCORES):
        o = results[c]["out2"]  # [2, DHP]
        out[c * NSH : c * NSH + DH, 0] = o[0, :DH]
        out[c * NSH + DH : (c + 1) * NSH, 0] = o[1, :DH]
    return out


def kernel(x, edge_index, W1, b1, W2, b2):
    global LAST_SIM_NS
    x = np.asarray(x, dtype=np.float32)
    edge_index = np.asarray(edge_index)
    inputs, meta = host_prep(x, edge_index, np.asarray(W1), b1, W2, b2)
    nc = build_fused(meta)
    if MEASURE:
        LAST_SIM_NS = _sim_ns(nc)
    res = run_bass_kernel_spmd(nc, inputs, list(range(NCORES)))
    return _assemble_out(res.results)


# revision 4
# speedup vs baseline: 1.0337x; 1.0063x over previous
"""2-layer GCN (PyG GCNConv x2 + sigmoid) on 8 TRN2 NeuronCores, single fused NEFF.

Cost-model-driven design (777965ns baseline -> 255916ns):
- ap_gather costs max(table_width, num_idxs)*0.833ns on GPSIMD; all gathers
  are sized so num_idxs >= table width (~0.833ns/edge-slot).
- Layer 1, dst-sharded: 8 src-octant tables [128, 6256] built by bf16
  matmuls (1 PE cycle/row; x uploaded pre-scaled by dinv[src] as bf16);
  per octant one ~14k-idx gather pass (2 chunks) + degree-ladder
  tensor_reduce (k=1 buckets are plain copies -> Activation engine).
- Assembly is software-pipelined one octant behind the gathers (accp double-
  buffered) so the in-order Pool queue never stalls on DVE reduces. Per
  octant: perm-gather to dst order, then nodes [0,3072) accumulate over
  octants in PSUM via identity matmul on the PE, the rest via DVE adds
  (PSUM cannot hold 6250 f32 cols next to the matmul staging bank).
- Finalize pipelines mul/sigmoid/z-matmul/z'-scale in 512-col chunks across
  DVE/Act/PE; z' rows are bf16 and AllGathered as one 12.5KB row pair per
  core (15us collective constant + payload at 40GB/s).
- Layer 2 is src-grouped: the 8 gpsimd 16-partition groups hold (src
  quarter, dst half) z' tables loaded bf16 in parallel on the 3 DMA queues
  and converted to f32 on Act+DVE; one gather covers all ~14k layer-2 slots;
  ladder reduce is split k=1->Act, k<=5->gpsimd strided adds, rest->DVE;
  cross-quarter combine is a 0/1-stationary PE matmul into [2, 3136];
  appended self-loops skip the gather entirely (their z'[n] term is the
  local z row, added in the finale). Dummy matmuls keep the PE p-state warm
  across gather windows. DMA cost is per-partition-bytes * 0.3855ns, so wide
  loads are split across the sync/scalar/gpsimd queues.
"""

import sys

sys.path.insert(0, "/opt/trn_rl_repo")
import numpy as np
from contextlib import ExitStack

from concourse import bacc, mybir
from concourse.tile import TileContext
from concourse.bass_utils import run_bass_kernel_spmd

try:
    import ml_dtypes

    _BF16 = np.dtype(ml_dtypes.bfloat16)
except Exception:  # pragma: no cover
    _BF16 = None

MEASURE = False
LAST_SIM_NS = None

N = 50000
E = 800000
F = 128
P = 128
NCORES = 8
NSH = N // NCORES  # 6250 dst nodes per core
NO = 8  # src octants (tables)
ON = N // NO  # 6250 src nodes per octant
OCOLS = 6256  # octant table cols: [zero, 6250 nodes, pad] mult of 16
PERM_NI = 6256  # pad16(NSH): assembly perm idx count per octant
H0 = 3072  # nodes 0..H0-1 accumulate in PSUM via PE
H1 = PERM_NI - H0  # 3200 (covers nodes H0..6249 + pad)
GMAX = 7808  # max L1 gather chunk (slots)
MMCH = 512  # matmul moving chunk
K2G = 8  # layer-2 partition groups = (src quarter, dst half)
DH = NSH // 2  # 3125 dst nodes per k2 half
DHP = 3136  # pad16(DH)
K2W = 4 * DHP + 16  # k2 table cols: [zero, 4 half-rows of 3136, pad]


def _wrap16(idx_flat):
    n = idx_flat.shape[0]
    assert n % 16 == 0
    return np.ascontiguousarray(idx_flat.reshape(n // 16, 16).T)


def _pad16(n, mult=16):
    return ((n + mult - 1) // mult) * mult


def _concat_aranges(lens):
    if len(lens) == 0:
        return np.zeros(0, dtype=np.int64)
    total = int(lens.sum())
    out = np.ones(total, dtype=np.int64)
    ends = np.cumsum(lens)
    out[0] = 0
    out[ends[:-1]] = -(lens[:-1] - 1)
    return np.cumsum(out)


def _bucket_lut(kmax, exact, buckets):
    lut = np.arange(max(kmax + 1, exact + 1))
    for kk in range(exact + 1, len(lut)):
        for bb in buckets:
            if kk <= bb:
                lut[kk] = bb
                break
        else:
            lut[kk] = ((kk + 63) // 64) * 64
    return lut


def _ladder_layout(kapb_all, cap_fn):
    """kapb_all: [NCORES, NCELLS, NNODES] bucketed kappas sharing one layout.
    Returns (budgets {k: n}, chunks list, descr [(ch, off, n_rows, k, col)],
    n_cols)."""
    b = {}
    for k in np.unique(kapb_all):
        k = int(k)
        if k == 0:
            continue
        nk = int((kapb_all == k).sum(axis=-1).max())
        if nk > 0:
            b[k] = nk
    raw = sum(k * n for k, n in b.items())
    cap = cap_fn(raw)
    descr = []
    col = 1
    ch, off = 0, 0
    for k in sorted(b):
        left = b[k]
        while left > 0:
            fit = min(left, (cap - off) // k)
            if fit == 0:
                ch += 1
                off = 0
                fit = min(left, cap // k)
            descr.append((ch, off, fit, k, col))
            off += fit * k
            col += fit
            left -= fit
    n_chunks = ch + 1
    return b, n_chunks, cap, descr, col


def _fill_slots(kv, lut, descr, col2k_base, s_sorted, cap):
    """Place each node's edges into its ladder row. kv: per-node actual count;
    s_sorted: edge values sorted by node. Returns (slot_positions, values,
    node_cols)."""
    kvb = lut[kv]
    nodes = np.nonzero(kv)[0]
    kn = kv[nodes]
    knb = kvb[nodes]
    nd = np.lexsort((nodes, knb))
    nodes_s, kn_s, knb_s = nodes[nd], kn[nd], knb[nd]
    rank = np.zeros(len(nodes_s), dtype=np.int64)
    colof = np.zeros(len(nodes_s), dtype=np.int64)
    for k in np.unique(knb_s):
        mk = knb_s == k
        rank[mk] = np.arange(mk.sum())
        colof[mk] = col2k_base[int(k)]
    node_col = colof + rank
    ncols = max(d[4] + d[2] for d in descr)
    col2slot = np.full(ncols, -1, dtype=np.int64)
    for ch, off, n_rows, k, col in descr:
        cols = np.arange(n_rows)
        col2slot[col + cols] = ch * cap + off + cols * k
    starts = col2slot[node_col]
    eslots = np.repeat(starts, kn_s) + _concat_aranges(kn_s)
    ptr = np.zeros(len(kv) + 1, dtype=np.int64)
    ptr[1:] = np.cumsum(kv)
    ev = (
        np.concatenate([s_sorted[ptr[n] : ptr[n + 1]] for n in nodes_s])
        if len(nodes_s)
        else np.zeros(0, dtype=np.int64)
    )
    pm = np.zeros(len(kv), dtype=np.int16)
    pm[nodes_s] = node_col.astype(np.int16)
    return eslots, ev, pm


def host_prep(x, edge_index, W1, b1, W2, b2):
    src = np.concatenate([edge_index[0], np.arange(N, dtype=np.int64)]).astype(np.int32)
    dst = np.concatenate([edge_index[1], np.arange(N, dtype=np.int64)]).astype(np.int32)
    deg = np.bincount(dst, minlength=N).astype(np.float32)
    dinv = 1.0 / np.sqrt(np.maximum(deg, 1e-12))
    dinv[deg <= 0] = 0.0

    # random node->table-position permutation balances per-(core,octant)
    # degree distributions, keeping shared max-over-core budgets tight
    psrc = np.random.default_rng(12345).permutation(N)
    pinv = np.argsort(psrc)

    xtp = (x * dinv[:, None]).T.astype(np.float32)[:, pinv]  # [128, N] pos order
    xt = np.zeros((P, NO * OCOLS), dtype=np.float32)
    for o in range(NO):
        xt[:, o * OCOLS + 1 : o * OCOLS + 1 + ON] = xtp[:, o * ON : (o + 1) * ON]
    xt_bf16 = xt.astype(_BF16)

    core = dst // NSH
    dstl = dst % NSH
    pos = psrc[src]
    octant = pos // ON
    srcl = (pos % ON).astype(np.int64) + 1

    # kappa per (core, octant, local dst node)
    kap = np.zeros((NCORES, NO, NSH), dtype=np.int32)
    for c in range(NCORES):
        mc = core == c
        for o in range(NO):
            m = mc & (octant == o)
            kap[c, o] = np.bincount(dstl[m], minlength=NSH)

    kmax = int(kap.max())
    lut = _bucket_lut(kmax, 12, (14, 16, 19, 22, 26, 32, 40, 48, 64, 96, 128, 192, 256))
    kapb = lut[kap]

    layouts = []  # per octant: (n_chunks, cap, descr, n_cols, kbase)
    for o in range(NO):
        b, n_chunks, cap, descr, ncol = _ladder_layout(
            kapb[:, o, :], lambda raw: min(GMAX, _pad16((raw + 1) // 2 + 64))
        )
        kbase = {}
        for ch, off, n_rows, k, col in descr:
            kbase.setdefault(k, col)
        layouts.append((n_chunks, cap, descr, ncol, kbase))

    SQ = [layouts[o][0] * layouts[o][1] for o in range(NO)]  # slots per octant
    PQ = _pad16(max(layouts[o][3] for o in range(NO)))

    order = np.lexsort((dstl, octant, core))
    so, do_, oo, co = srcl[order], dstl[order], octant[order], core[order]
    eidx = np.zeros((NCORES, sum(SQ)), dtype=np.int16)
    perms = np.zeros((NCORES, NO, PERM_NI), dtype=np.int16)
    for c in range(NCORES):
        obase = 0
        for o in range(NO):
            m = (co == c) & (oo == o)
            _, cap, descr, _, kbase = layouts[o]
            eslots, ev, pm = _fill_slots(kap[c, o], lut, descr, kbase, so[m], cap)
            eidx[c, obase + eslots] = ev.astype(np.int16)
            perms[c, o, :NSH] = pm
            obase += SQ[o]

    eidx_w = np.zeros((NCORES, P, sum(SQ) // 16), dtype=np.int16)
    perm_w = np.zeros((NCORES, P, NO * (PERM_NI // 16)), dtype=np.int16)
    for c in range(NCORES):
        eidx_w[c] = np.tile(_wrap16(eidx[c]), (K2G, 1))
        pw = np.concatenate([_wrap16(perms[c, o]) for o in range(NO)], axis=1)
        perm_w[c] = np.tile(pw, (K2G, 1))

    # ---------------- layer 2 (src-grouped) ----------------
    # Appended self-loops are excluded (their z'[n] term is added on-device
    # from the local z row); only the original E edges go through the gather.
    # group g = 2*src_quarter + dst_half; table per group: z' of src quarter
    # laid out as 4 half-rows of DHP (cores 2sq,2sq+1 x dst-halves), matching
    # the AllGather result zall [16, DHP].
    src2 = src[:E]
    dst2 = dst[:E]
    core2 = dst2 // NSH
    dstl2 = dst2 % NSH
    srcq = (src2 // (2 * NSH)).astype(np.int64)  # 0..3
    _c2 = (src2 // NSH).astype(np.int64) % 2  # core parity within quarter
    _i2 = (src2 % NSH).astype(np.int64)
    srcl2 = (2 * _c2 + _i2 // DH) * DHP + (_i2 % DH)  # table position (0-based)
    dh = dstl2 // DH  # 0..1
    j2 = dstl2 % DH  # 0..3124
    grp = 2 * srcq + dh

    kap2 = np.zeros((NCORES, K2G, DH), dtype=np.int32)
    for c in range(NCORES):
        mc = core2 == c
        for g in range(K2G):
            m = mc & (grp == g)
            kap2[c, g] = np.bincount(j2[m], minlength=DH)

    kmax2 = int(kap2.max())
    lut2 = _bucket_lut(kmax2, 9, (11, 13, 15, 18, 22, 27, 33, 40, 48, 64, 96, 128, 192, 256))
    kapb2 = lut2[kap2]
    b2_, n_chunks2, cap2, descr2, ncol2 = _ladder_layout(
        kapb2.reshape(NCORES, K2G, DH), lambda raw: _pad16(raw)
    )
    assert n_chunks2 == 1
    SQ2 = cap2
    P2 = _pad16(ncol2)
    kbase2 = {}
    for ch, off, n_rows, k, col in descr2:
        kbase2.setdefault(k, col)

    order2 = np.lexsort((j2, grp, core2))
    so2, jo2, go2, co2 = srcl2[order2], j2[order2], grp[order2], core2[order2]
    eidx2 = np.full((NCORES, K2G, SQ2), 4 * DHP, dtype=np.int16)  # pad -> zero col
    perm2 = np.zeros((NCORES, K2G, DHP), dtype=np.int16)
    for c in range(NCORES):
        for g in range(K2G):
            m = (co2 == c) & (go2 == g)
            eslots, ev, pm = _fill_slots(kap2[c, g], lut2, descr2, kbase2, so2[m], cap2)
            eidx2[c, g, eslots] = ev.astype(np.int16)
            perm2[c, g, :DH] = pm

    eidx2_w = np.zeros((NCORES, P, SQ2 // 16), dtype=np.int16)
    perm2_w = np.zeros((NCORES, P, DHP // 16), dtype=np.int16)
    for c in range(NCORES):
        for g in range(K2G):
            eidx2_w[c, g * 16 : (g + 1) * 16] = _wrap16(eidx2[c, g])
            perm2_w[c, g * 16 : (g + 1) * 16] = _wrap16(perm2[c, g])

    ident = np.eye(P, dtype=np.float32)
    wones = np.zeros((P, 2), dtype=np.float32)
    for g in range(K2G):
        wones[16 * g, g % 2] = 1.0

    dinvb = np.zeros((NCORES, P, NSH), dtype=np.float32)
    dinvd2 = np.zeros((NCORES, 2, DHP), dtype=np.float32)
    for c in range(NCORES):
        dsh = dinv[c * NSH : (c + 1) * NSH]
        dinvb[c] = np.tile(dsh, (P, 1))
        dinvd2[c, 0, :DH] = dsh[:DH]
        dinvd2[c, 1, :DH] = dsh[DH:]

    meta = dict(layouts=layouts, SQ=SQ, PQ=PQ, descr2=descr2, SQ2=SQ2, P2=P2,
                b2=float(np.asarray(b2).reshape(-1)[0]))
    inputs = []
    for c in range(NCORES):
        inputs.append(
            {
                "xt": xt_bf16,
                "w1": W1.astype(np.float32).astype(_BF16),
                "b1": np.asarray(b1, dtype=np.float32).reshape(P, 1),
                "w2": np.asarray(W2, dtype=np.float32).reshape(P, 1),
                "ident": ident,
                "wones": wones,
                "eidx": np.ascontiguousarray(eidx_w[c]),
                "perm": np.ascontiguousarray(perm_w[c]),
                "eidx2": np.ascontiguousarray(eidx2_w[c]),
                "perm2": np.ascontiguousarray(perm2_w[c]),
                "dinvb": np.ascontiguousarray(dinvb[c]),
                "dinvd2": np.ascontiguousarray(dinvd2[c]),
            }
        )
    return inputs, meta


def build_fused(meta):
    layouts, SQ, PQ = meta["layouts"], meta["SQ"], meta["PQ"]
    descr2, SQ2, P2 = meta["descr2"], meta["SQ2"], meta["P2"]
    b2 = meta["b2"]
    nc = bacc.Bacc(None, target_bir_lowering=False)
    f32, bf16, i16 = mybir.dt.float32, mybir.dt.bfloat16, mybir.dt.int16

    xt_d = nc.dram_tensor("xt", [P, NO * OCOLS], bf16, kind="ExternalInput")
    w1_d = nc.dram_tensor("w1", [P, P], bf16, kind="ExternalInput")
    b1_d = nc.dram_tensor("b1", [P, 1], f32, kind="ExternalInput")
    w2_d = nc.dram_tensor("w2", [P, 1], f32, kind="ExternalInput")
    ident_d = nc.dram_tensor("ident", [P, P], f32, kind="ExternalInput")
    wones_d = nc.dram_tensor("wones", [P, 2], f32, kind="ExternalInput")
    eidx_d = nc.dram_tensor("eidx", [P, sum(SQ) // 16], i16, kind="ExternalInput")
    perm_d = nc.dram_tensor("perm", [P, NO * (PERM_NI // 16)], i16, kind="ExternalInput")
    eidx2_d = nc.dram_tensor("eidx2", [P, SQ2 // 16], i16, kind="ExternalInput")
    perm2_d = nc.dram_tensor("perm2", [P, DHP // 16], i16, kind="ExternalInput")
    dinvb_d = nc.dram_tensor("dinvb", [P, NSH], f32, kind="ExternalInput")
    dinvd2_d = nc.dram_tensor("dinvd2", [2, DHP], f32, kind="ExternalInput")
    out_d = nc.dram_tensor("out2", [2, DHP], f32, kind="ExternalOutput")
    zin = nc.dram_tensor("zin_cc", [2, DHP], bf16, kind="Internal")
    zall = nc.dram_tensor("zall_cc", [4, 4 * DHP], bf16, kind="Internal", addr_space="Shared")

    with ExitStack() as ctx:
        tc = ctx.enter_context(TileContext(nc))
        cpool = ctx.enter_context(tc.tile_pool(name="cpool", bufs=1))
        w1 = cpool.tile([P, P], bf16)
        b1 = cpool.tile([P, 1], f32)
        w2 = cpool.tile([P, 1], f32)
        ident = cpool.tile([P, P], f32)
        wones = cpool.tile([P, 2], f32)
        perm = cpool.tile([P, NO * (PERM_NI // 16)], i16)
        nc.scalar.dma_start(out=w1[:], in_=w1_d[:])
        nc.scalar.dma_start(out=b1[:], in_=b1_d[:])
        nc.scalar.dma_start(out=w2[:], in_=w2_d[:])
        nc.scalar.dma_start(out=ident[:], in_=ident_d[:])
        nc.scalar.dma_start(out=wones[:], in_=wones_d[:])
        nc.scalar.dma_start(out=perm[:], in_=perm_d[:])


        with (
            tc.tile_pool(name="apool", bufs=1) as apool,
            tc.tile_pool(name="pshpool", bufs=1, space="PSUM") as pshpool,
        ):
            acc1 = apool.tile([P, H1], f32)
            ps_h0 = pshpool.tile([P, H0], f32)

            with (
                tc.tile_pool(name="tabs", bufs=2) as tabs,
                tc.tile_pool(name="xpool", bufs=2) as xpool,
                tc.tile_pool(name="epool", bufs=2) as epool,
                tc.tile_pool(name="gpool", bufs=2) as gpool,
                tc.tile_pool(name="appool", bufs=2) as appool,
                tc.tile_pool(name="tpool", bufs=1) as tpool,
                tc.tile_pool(name="pstab", bufs=1, space="PSUM") as pstab,
            ):
                sq_base = 0
                SQMAX = max(SQ)
                CAPMAX = max(layouts[o][1] for o in range(NO))
                prev_accp = None  # assembly for octant o runs after gathers o+1

                def _assemble(o, accp_o):
                    # perm-gather accp -> dst order, then accumulate: nodes
                    # [0,H0) into PSUM via identity matmul, [H0,) via DVE adds
                    if o == NO - 1:
                        # keep PE warm across the reduce+perm wait so the
                        # final assembly matmuls run at full clock
                        pst = pstab.tile([P, 2 * MMCH], f32, tag="pstab")
                        for i in range(5):
                            nc.tensor.matmul(
                                pst[0:1, :MMCH], w2[:],
                                accp_o[:, (i % 8) * MMCH : (i % 8 + 1) * MMCH],
                                start=True, stop=True,
                            )
                    t = tpool.tile([P, PERM_NI], f32, tag="t")
                    pbase = o * (PERM_NI // 16)
                    nc.gpsimd.ap_gather(
                        t[:], accp_o[:], perm[:, pbase : pbase + PERM_NI // 16],
                        channels=P, num_elems=PQ, d=1, num_idxs=PERM_NI,
                    )
                    for m0 in range(0, H0, MMCH):
                        nc.tensor.matmul(
                            ps_h0[:, m0 : m0 + MMCH], ident[:], t[:, m0 : m0 + MMCH],
                            start=(o == 0), stop=(o == NO - 1), skip_group_check=True,
                        )
                    if o == 0:
                        nc.scalar.activation(
                            acc1[:], t[:, H0:], mybir.ActivationFunctionType.Copy
                        )
                    else:
                        nc.vector.tensor_add(acc1[:], acc1[:], t[:, H0:])

                for o in range(NO):
                    n_chunks, cap, descr, _, _ = layouts[o]
                    tab = tabs.tile([P, OCOLS], f32, tag="tab")
                    XB = 1568 if o == 0 else 784
                    for x0 in range(0, OCOLS, XB):
                        xw = min(XB, OCOLS - x0)
                        xc = xpool.tile([P, 1568 if o == 0 else 784], bf16, tag="x")
                        nc.sync.dma_start(
                            out=xc[:, :xw], in_=xt_d[:, o * OCOLS + x0 : o * OCOLS + x0 + xw]
                        )
                        for m0 in range(0, xw, 2 * MMCH):
                            mw = min(2 * MMCH, xw - m0)
                            ps = pstab.tile([P, 2 * MMCH], f32, tag="pstab")
                            for h in range(0, mw, MMCH):
                                hw_ = min(MMCH, mw - h)
                                nc.tensor.matmul(
                                    ps[:, h : h + hw_], w1[:], xc[:, m0 + h : m0 + h + hw_],
                                    start=True, stop=True,
                                )
                            nc.scalar.activation(
                                tab[:, x0 + m0 : x0 + m0 + mw], ps[:, :mw],
                                mybir.ActivationFunctionType.Copy,
                            )
                    # gather + ladder reduce (k=1 buckets are copies -> Act)
                    eb = epool.tile([P, SQMAX // 16], i16, tag="eidx")
                    (nc.gpsimd if o == 0 else nc.sync).dma_start(
                        out=eb[:, : SQ[o] // 16],
                        in_=eidx_d[:, sq_base // 16 : (sq_base + SQ[o]) // 16],
                    )
                    accp = appool.tile([P, PQ], f32, tag="accp")
                    nc.vector.memset(accp[:, 0:1], 0.0)
                    by_chunk = {}
                    for d_ in descr:
                        by_chunk.setdefault(d_[0], []).append(d_)
                    for ch in range(n_chunks):
                        g = gpool.tile([P, CAPMAX], f32, tag="g")
                        nc.gpsimd.ap_gather(
                            g[:, :cap], tab[:], eb[:, ch * cap // 16 : (ch + 1) * cap // 16],
                            channels=P, num_elems=OCOLS, d=1, num_idxs=cap,
                        )
                        for (_, off, n_rows, k, col) in by_chunk.get(ch, []):
                            if k == 1:
                                nc.scalar.activation(
                                    accp[:, col : col + n_rows], g[:, off : off + n_rows],
                                    mybir.ActivationFunctionType.Copy,
                                )
                            elif k <= 4:
                                # tensor_add is charged by output size, reduce
                                # by input size: strided adds win for small k
                                s3 = g[:, off : off + n_rows * k].rearrange(
                                    "p (a b) -> p a b", a=n_rows, b=k
                                )
                                dstp = accp[:, col : col + n_rows]
                                nc.vector.tensor_add(dstp, s3[:, :, 0], s3[:, :, 1])
                                for j in range(2, k):
                                    nc.vector.tensor_add(dstp, dstp, s3[:, :, j])
                            else:
                                nc.vector.tensor_reduce(
                                    accp[:, col : col + n_rows],
                                    g[:, off : off + n_rows * k].rearrange(
                                        "p (a b) -> p a b", a=n_rows, b=k
                                    ),
                                    axis=mybir.AxisListType.X, op=mybir.AluOpType.add,
                                )
                    if prev_accp is not None:
                        _assemble(o - 1, prev_accp)
                    prev_accp = accp
                    sq_base += SQ[o]
                _assemble(NO - 1, prev_accp)

            # finalize layer 1: sigma1 = sigmoid(dinv*acc + b1); z' = dinv * W2^T sigma1
            # z' is kept as two 1-partition rows (dst halves at DH=3125, padded
            # to DHP), scaled by dinvd2 on write, then DMA'd to zin [2, DHP]
            with (
                tc.tile_pool(name="fin", bufs=1) as fin,
                tc.tile_pool(name="psz", bufs=2, space="PSUM") as psz,
            ):
                s0 = fin.tile([P, H0], f32)
                s1 = fin.tile([P, NSH - H0], f32)
                dinvb = fin.tile([P, NSH], f32)
                zr0 = fin.tile([1, DHP], bf16)
                zr1 = fin.tile([1, DHP], bf16)
                d20 = fin.tile([1, DHP], f32)
                d21 = fin.tile([1, DHP], f32)
                nc.scalar.dma_start(out=dinvb[:, H0:], in_=dinvb_d[:, H0:])
                nc.sync.dma_start(out=dinvb[:, :H0], in_=dinvb_d[:, :H0])
                nc.sync.dma_start(out=d20[:], in_=dinvd2_d[0:1, :])
                nc.sync.dma_start(out=d21[:], in_=dinvd2_d[1:2, :])
                nc.vector.memset(zr0[:, DH:], 0.0)
                nc.vector.memset(zr1[:, DH:], 0.0)

                def _zr_write(ps, glob0, w):
                    # ps[0, :w] holds z~ for global nodes [glob0, glob0+w)
                    done = 0
                    while done < w:
                        g0 = glob0 + done
                        dh_ = g0 // DH
                        j0 = g0 - dh_ * DH
                        n = min(w - done, DH - j0)
                        zrt = zr1 if dh_ else zr0
                        d2t = d21 if dh_ else d20
                        nc.vector.tensor_mul(
                            zrt[:, j0 : j0 + n], ps[:, done : done + n],
                            d2t[:, j0 : j0 + n],
                        )
                        done += n

                # drain both halves in MMCH chunks so mul/sigmoid/z-matmul/
                # zr-mul pipeline across engines
                for m0 in range(0, NSH - H0, MMCH):
                    w = min(MMCH, NSH - H0 - m0)
                    sl = slice(m0, m0 + w)
                    nc.vector.tensor_mul(s1[:, sl], acc1[:, sl], dinvb[:, H0 + m0 : H0 + m0 + w])
                    nc.scalar.activation(
                        s1[:, sl], s1[:, sl], mybir.ActivationFunctionType.Sigmoid,
                        bias=b1[:, 0:1],
                    )
                    ps = psz.tile([1, MMCH], f32, tag="psz")
                    nc.tensor.matmul(
                        ps[:, :w], w2[:], s1[:, sl], start=True, stop=True
                    )
                    _zr_write(ps, H0 + m0, w)
                for m0 in range(0, H0, MMCH):
                    sl = slice(m0, m0 + MMCH)
                    nc.vector.tensor_mul(s0[:, sl], ps_h0[:, sl], dinvb[:, sl])
                    nc.scalar.activation(
                        s0[:, sl], s0[:, sl], mybir.ActivationFunctionType.Sigmoid,
                        bias=b1[:, 0:1],
                    )
                    ps = psz.tile([1, MMCH], f32, tag="psz")
                    nc.tensor.matmul(ps[:], w2[:], s0[:, sl], start=True, stop=True)
                    _zr_write(ps, m0, MMCH)
                nc.sync.dma_start(out=zin[0:1, :], in_=zr0[:])
                nc.scalar.dma_start(out=zin[1:2, :], in_=zr1[:])

        nc.gpsimd.collective_compute(
            "AllGather", mybir.AluOpType.bypass,
            replica_groups=[list(range(NCORES))],
            ins=[zin[:].opt()], outs=[zall[:].opt()],
        )

        # ---- layer 2 ----
        with (
            tc.tile_pool(name="k2pool", bufs=1) as pool2,
            tc.tile_pool(name="ps2pool", bufs=1, space="PSUM") as ps2pool,
        ):
            zt = pool2.tile([P, K2W], f32)
            ztb = pool2.tile([P, K2W], bf16)
            g2 = pool2.tile([P, SQ2], f32)
            eidx2 = pool2.tile([P, SQ2 // 16], i16)
            perm2 = pool2.tile([P, DHP // 16], i16)
            accp2 = pool2.tile([P, P2], f32)
            t2 = pool2.tile([P, DHP], f32)
            o2 = pool2.tile([2, DHP], f32)
            zrTb = pool2.tile([2, DHP], bf16)
            zrT = pool2.tile([2, DHP], f32)
            dinvd2T = pool2.tile([2, DHP], f32)
            ps2 = ps2pool.tile([2, DHP], f32)
            # zero ztb in full (garbage partitions must be 0.0, not junk, so
            # the 0-weighted rows of the wones matmul cannot poison the sum);
            # runs on Act during the collective window
            nc.scalar.memzero(ztb[:])
            nc.vector.memset(accp2[:, 0:1], 0.0)
            # local z' + dst dinv in [2, DHP] layout for the final combine
            # (zin round-trip; overlaps the collective)
            nc.scalar.dma_start(out=zrTb[:], in_=zin[:])
            nc.scalar.dma_start(out=dinvd2T[:], in_=dinvd2_d[:])
            nc.scalar.dma_start(out=eidx2[:], in_=eidx2_d[:])
            nc.scalar.dma_start(out=perm2[:], in_=perm2_d[:])
            nc.scalar.activation(zrT[:], zrTb[:], mybir.ActivationFunctionType.Copy)
            # group 2*sq + dh holds z' of src quarter sq (dh in {0,1} share it),
            # loaded as bf16 [4, 4*DHP] from zall (zero col lives at 4*DHP);
            # split into col chunks over the 3 DMA-capable queues, then
            # converted to the f32 gather table (Act + DVE halves)
            ZC = ((4 * DHP + 2) // 3 + 15) // 16 * 16  # 4192: 3 col chunks
            zq = [nc.sync, nc.scalar, nc.gpsimd]
            for i in range(3):
                c0 = i * ZC
                cw = min(ZC, 4 * DHP - c0)
                zq[i].dma_start(
                    out=ztb[0:P:32, c0 : c0 + cw], in_=zall[:, c0 : c0 + cw]
                )
                zq[(i + 1) % 3].dma_start(
                    out=ztb[16:P:32, c0 : c0 + cw], in_=zall[:, c0 : c0 + cw]
                )
            for i in range(3):
                c0 = i * ZC
                cw = (min(ZC, K2W - c0) if i < 2 else K2W - c0)
                if i < 2:
                    nc.scalar.activation(
                        zt[:, c0 : c0 + cw], ztb[:, c0 : c0 + cw],
                        mybir.ActivationFunctionType.Copy,
                    )
                else:
                    nc.vector.tensor_copy(zt[:, c0:], ztb[:, c0:])
            nc.gpsimd.ap_gather(
                g2[:], zt[:], eidx2[:], channels=P, num_elems=K2W, d=1, num_idxs=SQ2
            )
            # ladder reduce, split across engines: k=1 -> Act copy, small k ->
            # gpsimd strided adds (Pool is idle here), rest -> DVE reduce
            for (_, off, n_rows, k, col) in descr2:
                dst_ap = accp2[:, col : col + n_rows]
                if k == 1:
                    nc.scalar.activation(
                        dst_ap, g2[:, off : off + n_rows],
                        mybir.ActivationFunctionType.Copy,
                    )
                elif k in (2, 3, 4, 5, 6):
                    src3 = g2[:, off : off + n_rows * k].rearrange(
                        "p (a b) -> p a b", a=n_rows, b=k
                    )
                    nc.gpsimd.tensor_add(dst_ap, src3[:, :, 0], src3[:, :, 1])
                    for j in range(2, k):
                        nc.gpsimd.tensor_add(dst_ap, dst_ap, src3[:, :, j])
                else:
                    nc.vector.tensor_reduce(
                        dst_ap,
                        g2[:, off : off + n_rows * k].rearrange(
                            "p (a b) -> p a b", a=n_rows, b=k
                        ),
                        axis=mybir.AxisListType.X, op=mybir.AluOpType.add,
                    )
            # keep the PE p-state warm through the gather/reduce window so the
            # real matmuls below run at full clock: dummies chain on the zt
            # load, then on reduce outputs (accp2 slices) to span the window
            psd = ps2pool.tile([1, MMCH], f32)
            for i in range(4):
                nc.tensor.matmul(
                    psd[:], w2[:], zt[:, i * MMCH : (i + 1) * MMCH],
                    start=True, stop=True,
                )
            for i in range(9):
                nc.tensor.matmul(
                    psd[:], w2[:], accp2[:, (i % 6) * MMCH : (i % 6 + 1) * MMCH],
                    start=True, stop=True,
                )
            nc.gpsimd.ap_gather(
                t2[:], accp2[:], perm2[:], channels=P, num_elems=P2, d=1, num_idxs=DHP
            )
            # out2 = sigmoid(dinv_dst * (gathered sum + local self-loop z') + b2)
            # chunked so add/mul/sigmoid/DMA pipeline behind the matmuls
            for m0 in range(0, DHP, MMCH):
                w = min(MMCH, DHP - m0)
                sl = slice(m0, m0 + w)
                nc.tensor.matmul(
                    ps2[:, sl], wones[:], t2[:, sl], start=True, stop=True
                )
                nc.vector.tensor_add(o2[:, sl], ps2[:, sl], zrT[:, sl])
                nc.vector.tensor_mul(o2[:, sl], o2[:, sl], dinvd2T[:, sl])
                nc.scalar.activation(
                    o2[:, sl], o2[:, sl], mybir.ActivationFunctionType.Sigmoid, bias=b2
                )
                (nc.sync if (m0 // MMCH) % 2 == 0 else nc.scalar).dma_start(
                    out=out_d[:, sl], in_=o2[:, sl]
                )
    nc.finalize()
    return nc


def _sim_ns(nc):
    from concourse import bass_interp

    sim = bass_interp.CoreSim(nc, no_exec=True, publish_trace=False)
    sim.simulate()
    return int(sim.time)


def _assemble_out(results):
    out = np.zeros((N, 1), dtype=np.float32)
    for c in range(NCORES):
        o = results[c]["out2"]  # [2, DHP]
        out[c * NSH : c * NSH + DH, 0] = o[0, :DH]
        out[c * NSH + DH : (c + 1) * NSH, 0] = o[1, :DH]
    return out


def kernel(x, edge_index, W1, b1, W2, b2):
    global LAST_SIM_NS
    x = np.asarray(x, dtype=np.float32)
    edge_index = np.asarray(edge_index)
    inputs, meta = host_prep(x, edge_index, np.asarray(W1), b1, W2, b2)
    nc = build_fused(meta)
    if MEASURE:
        LAST_SIM_NS = _sim_ns(nc)
    res = run_bass_kernel_spmd(nc, inputs, list(range(NCORES)))
    return _assemble_out(res.results)
